# revision 1
# baseline (speedup 1.0000x reference)
"""Trainium2 Bass kernel for nn_CNN_80221399155117.

Pipeline: full-vocab softmax -> token-prob gather -> -log2 surprisal ->
concat(hidden, surp) -> Conv1d(k=5, pad=2) -> MaxPool1d(5) -> ReLU -> FC.

Sharding: 8 cores = (batch b, seq-half h). Each core owns the pool-aligned
conv-output range [510h, 510h+510) of its batch, needing feats rows
[510h-2, 510h+512) (EXT=514, zero-padded outside [0,1024)). The softmax
normalizer is computed locally per row (positions sharded, vocab local),
so no collectives are needed. The token-logit gather runs on-device via
indirect DMA with flat indices built from iota + input_ids.
"""

import numpy as np

B, S, V, H = 4, 1024, 32000, 2048
OC, K = 128, 5
N_CORES = 8
Y_LOC = 510            # conv output positions per core (102 pool windows)
PO_LOC = 102           # pooled cols per core
EXT = 514              # feats rows incl conv halo (510 + 2 + 2)
CF = 4000              # vocab chunk (free-dim) size
NCH = V // CF          # 8 chunks
LOG2E = 1.4426950408889634

_CACHE = {}
VARIANT = "indirect"   # bisect knob: indirect | nogather | flat2d | nopass1 | noconv


def _build_program():
    import concourse.tile as tile
    from concourse import bacc, bass, mybir
    from concourse.masks import make_identity

    f32 = mybir.dt.float32
    i32 = mybir.dt.int32
    Alu = mybir.AluOpType
    Act = mybir.ActivationFunctionType

    nc = bacc.Bacc("TRN2", target_bir_lowering=False, debug=False,
                   num_devices=N_CORES)

    logits = nc.dram_tensor("logits_loc", [EXT, V], f32, kind="ExternalInput").ap()
    ids = nc.dram_tensor("ids_loc", [EXT, 1], i32, kind="ExternalInput").ap()
    maskd = nc.dram_tensor("mask_loc", [EXT, 1], f32, kind="ExternalInput").ap()
    hid = nc.dram_tensor("hidden_loc", [EXT, H], f32, kind="ExternalInput").ap()
    wt = nc.dram_tensor("wt", [H, K * OC], f32, kind="ExternalInput").ap()
    wsurp = nc.dram_tensor("wsurp", [K, OC], f32, kind="ExternalInput").ap()
    convb = nc.dram_tensor("convb", [OC, 1], f32, kind="ExternalInput").ap()
    fcw = nc.dram_tensor("fcw", [OC, 3 * PO_LOC], f32, kind="ExternalInput").ap()
    sentv = nc.dram_tensor("sentv", [128, 1], f32, kind="ExternalInput").ap()
    sentw = nc.dram_tensor("sentw", [128, 3], f32, kind="ExternalInput").ap()
    fcb = nc.dram_tensor("fcb", [3, 1], f32, kind="ExternalInput").ap()
    out = nc.dram_tensor("out_loc", [3, 1], f32, kind="ExternalOutput").ap()

    surp_dram = nc.dram_tensor("surp_scratch", [1, EXT], f32).ap()

    logits_flat = bass.AP(logits.tensor, 0, [[1, EXT * V], [1, 1]])

    ROW_TILES = [(0, 128), (128, 128), (256, 128), (384, 128)]
    NHALO = EXT - 512                  # 2 halo rows, packed [128, HF]
    HQ = 128 // NHALO                  # partitions per halo row
    HF = V // HQ                       # free elems per partition

    with tile.TileContext(nc) as tc:
        with (
            tc.tile_pool(name="lp", bufs=6) as lp,          # logits chunks
            tc.tile_pool(name="scr", bufs=2) as scr,        # exp scratch
            tc.tile_pool(name="big", bufs=1) as big,        # resident X / weights
            tc.tile_pool(name="hn", bufs=2) as hnp,         # hidden natural tiles
            tc.tile_pool(name="sm", bufs=12) as sm,         # small per-tile stats
            tc.tile_pool(name="ps_t", bufs=4, space="PSUM") as ps_t,
            tc.tile_pool(name="ps_y", bufs=1, space="PSUM") as ps_y,
            tc.tile_pool(name="ps_o", bufs=1, space="PSUM") as ps_o,
        ):
            # ---- resident constants ----
            ident = big.tile([128, 128], f32, tag="ident")
            make_identity(nc, ident[:])
            f16 = mybir.dt.float16
            wtile = big.tile([128, 16 * K * OC], f16, tag="wtile")  # 16 ch-chunks
            for cc in range(16):
                nc.gpsimd.dma_start(        # SWDGE casts f32->bf16 in flight
                    out=wtile[:, cc * 640:(cc + 1) * 640],
                    in_=wt[cc * 128:(cc + 1) * 128, :],
                )
            wsurp_sb = big.tile([K, OC], f32, tag="wsurp")
            nc.sync.dma_start(out=wsurp_sb[:], in_=wsurp)
            convb_sb = big.tile([OC, 1], f32, tag="convb")
            nc.sync.dma_start(out=convb_sb[:], in_=convb)
            fcw_sb = big.tile([OC, 3 * PO_LOC], f32, tag="fcw")
            nc.sync.dma_start(out=fcw_sb[:], in_=fcw)
            sentv_sb = big.tile([128, 1], f32, tag="sentv")
            nc.sync.dma_start(out=sentv_sb[:], in_=sentv)
            sentw_sb = big.tile([128, 3], f32, tag="sentw")
            nc.sync.dma_start(out=sentw_sb[:], in_=sentw)
            fcb_sb = big.tile([3, 1], f32, tag="fcb")
            nc.sync.dma_start(out=fcb_sb[:], in_=fcb)
            ones_sb = big.tile([128, 1], f32, tag="ones")
            nc.vector.memset(ones_sb[:], 1.0)

            # ---- hidden -> transposed X tiles [ch, pos] ----
            xt = big.tile([128, 16 * EXT], f16, tag="xt")
            for r0, pn in ROW_TILES + [(512, NHALO)]:
                hn = hnp.tile([128, H], f32, tag="hn")
                nc.sync.dma_start(out=hn[:pn, :], in_=hid[r0:r0 + pn, :])
                for cc in range(16):
                    tp = ps_t.tile([128, 128], f32, tag="tp")
                    nc.tensor.transpose(
                        out=tp[:, :pn],
                        in_=hn[:pn, cc * 128:(cc + 1) * 128],
                        identity=ident[:pn, :pn],
                    )
                    nc.vector.tensor_copy(
                        out=xt[:, cc * EXT + r0: cc * EXT + r0 + pn],
                        in_=tp[:, :pn],
                    )

            # ---- conv: 80 hidden matmuls accumulate into one PSUM bank ----
            psum_y = ps_y.tile([OC, Y_LOC], f32, tag="y")
            first = True
            for cc in range(16):
                for k in range(K):
                    nc.tensor.matmul(
                        out=psum_y[:],
                        lhsT=wtile[:, cc * 640 + k * 128: cc * 640 + (k + 1) * 128],
                        rhs=xt[:, cc * EXT + k: cc * EXT + k + Y_LOC],
                        start=first,
                        stop=False,
                    )
                    first = False

            # ---- pass-1 shared stats, gathered upfront ----
            # cols 0..3 = main row tiles (row = 128*t + p), col 4 = halo rows
            NT = len(ROW_TILES)
            se_all = big.tile([128, NT + 1], f32, tag="se")    # sum(exp)
            g_all = big.tile([128, NT + 1], f32, tag="g")      # gathered logit
            m_all = big.tile([128, NT + 1], f32, tag="m")      # attention mask
            nc.vector.memset(se_all[:, NT:], 1.0)   # ln(1)=0 on unused lanes
            nc.vector.memset(g_all[:, NT:], 0.0)
            nc.vector.memset(m_all[:, NT:], 0.0)

            ids_all = sm.tile([128, NT], i32, tag="ids")
            nc.gpsimd.dma_start(out=ids_all[:],
                                in_=bass.AP(ids.tensor, 0, [[1, 128], [128, NT]]))
            nc.gpsimd.dma_start(out=m_all[:, :NT],
                                in_=bass.AP(maskd.tensor, 0, [[1, 128], [128, NT]]))
            nc.gpsimd.dma_start(out=m_all[:NHALO, NT:], in_=maskd[512:EXT, :])
            iota_t = sm.tile([128, NT], i32, tag="iota")
            nc.gpsimd.iota(iota_t[:], pattern=[[1, NT]], base=0,
                           channel_multiplier=0)
            nc.vector.tensor_scalar(out=iota_t[:], in0=iota_t[:],
                                    scalar1=128 * V, scalar2=None, op0=Alu.mult)
            iota_p = sm.tile([128, 1], i32, tag="iotap")
            nc.gpsimd.iota(iota_p[:], pattern=[[1, 1]], base=0,
                           channel_multiplier=V)
            flat_all = sm.tile([128, NT], i32, tag="flat")
            nc.vector.tensor_tensor(out=flat_all[:], in0=ids_all[:],
                                    in1=iota_t[:], op=Alu.add)
            nc.vector.tensor_tensor(out=flat_all[:], in0=flat_all[:],
                                    in1=iota_p[:].to_broadcast([128, NT]),
                                    op=Alu.add)
            for t in range(NT):
                # HW DGE honors only one index per partition per transfer
                nc.gpsimd.indirect_dma_start(
                    out=g_all[:, t:t + 1], out_offset=None, in_=logits_flat,
                    in_offset=bass.IndirectOffsetOnAxis(
                        ap=flat_all[:, t:t + 1], axis=0))
            # halo gather
            hrb = sm.tile([128, 1], i32, tag="hrb")
            nc.gpsimd.iota(hrb[:NHALO, :], pattern=[[1, 1]], base=512 * V,
                           channel_multiplier=V)
            hids = sm.tile([128, 1], i32, tag="hids")
            nc.gpsimd.dma_start(out=hids[:NHALO, :], in_=ids[512:EXT, :])
            hfl = sm.tile([128, 1], i32, tag="hfl")
            nc.vector.tensor_tensor(out=hfl[:NHALO, :], in0=hids[:NHALO, :],
                                    in1=hrb[:NHALO, :], op=Alu.add)
            nc.gpsimd.indirect_dma_start(
                out=g_all[:NHALO, NT:], out_offset=None, in_=logits_flat,
                in_offset=bass.IndirectOffsetOnAxis(ap=hfl[:NHALO, :1], axis=0))

            # ---- halo rows (2): vocab packed across partitions ----
            # layout [128, HF]: partition p = (row a=p//HQ, slice q=p%HQ)
            hx = lp.tile([128, HF], f32, tag="x")
            halo_src = bass.AP(logits.tensor, 512 * V,
                               [[V, NHALO], [HF, HQ], [1, HF]])
            nc.sync.dma_start(out=hx[:], in_=halo_src)
            hscr = scr.tile([128, HF], f32, tag="e")
            hsums = sm.tile([128, 1], f32, tag="hsums")
            nc.scalar.activation(out=hscr[:], in_=hx[:], func=Act.Exp,
                                 accum_out=hsums[:])
            hsel = big.tile([128, NHALO], f32, tag="hsel")
            nc.vector.memset(hsel[:], 0.0)
            for a in range(NHALO):
                nc.vector.memset(hsel[a * HQ:(a + 1) * HQ, a:a + 1], 1.0)
            psum_h = ps_o.tile([NHALO, 1], f32, tag="ph")
            nc.tensor.matmul(out=psum_h[:], lhsT=hsel[:], rhs=hsums[:],
                             start=True, stop=True)
            nc.vector.tensor_copy(out=se_all[:NHALO, NT:], in_=psum_h[:])

            # ---- pass 1: 8 exp chunks per main row tile, nothing else ----
            for t, (r0, pn) in enumerate(ROW_TILES):
                sums = sm.tile([128, NCH], f32, tag="sums")
                for ci in range(NCH):
                    x_sb = lp.tile([128, CF], f32, tag="x")
                    nc.sync.dma_start(
                        out=x_sb[:pn, :],
                        in_=logits[r0:r0 + pn, ci * CF:(ci + 1) * CF],
                    )
                    e_sb = scr.tile([128, CF], f32, tag="e")
                    nc.scalar.activation(
                        out=e_sb[:pn, :], in_=x_sb[:pn, :], func=Act.Exp,
                        accum_out=sums[:pn, ci:ci + 1],
                    )
                nc.vector.tensor_reduce(
                    out=se_all[:, t:t + 1], in_=sums[:, :],
                    axis=mybir.AxisListType.X, op=Alu.add,
                )

            # ---- batched LSE -> surp -> srow ----
            lse_all = sm.tile([128, NT + 1], f32, tag="lse")
            nc.scalar.activation(out=lse_all[:], in_=se_all[:], func=Act.Ln)
            surp_all = sm.tile([128, NT + 1], f32, tag="surp")
            nc.vector.tensor_tensor(out=surp_all[:], in0=lse_all[:],
                                    in1=g_all[:], op=Alu.subtract)
            nc.vector.tensor_tensor(out=surp_all[:], in0=surp_all[:],
                                    in1=m_all[:], op=Alu.mult)
            nc.vector.tensor_scalar(out=surp_all[:], in0=surp_all[:],
                                    scalar1=LOG2E, scalar2=None, op0=Alu.mult)
            srow = big.tile([1, EXT], f32, tag="srow")
            for t in range(NT):
                nc.gpsimd.dma_start(out=srow[0:1, 128 * t:128 * (t + 1)],
                                    in_=surp_all[:, t:t + 1])
            nc.gpsimd.dma_start(out=srow[0:1, 512:EXT],
                                in_=surp_all[:NHALO, NT:])

            # ---- surp channel: one contract-5 matmul closes the accumulation ----
            s5 = big.tile([K, Y_LOC], f32, tag="s5")
            for k in range(K):
                nc.gpsimd.dma_start(out=s5[k:k + 1, :],
                                    in_=srow[0:1, k:k + Y_LOC])
            nc.tensor.matmul(
                out=psum_y[:],
                lhsT=wsurp_sb[:],
                rhs=s5[:],
                start=False,
                stop=True,
            )

            # ---- maxpool(5) + bias + relu ----
            pooled = big.tile([OC, PO_LOC], f32, tag="pooled")
            stop_off = K * (PO_LOC - 1) + 1
            nc.vector.tensor_copy(out=pooled[:], in_=psum_y[:, 0:stop_off:K])
            for j in range(1, K):
                nc.vector.tensor_tensor(out=pooled[:], in0=pooled[:],
                                        in1=psum_y[:, j:j + stop_off:K], op=Alu.max)
            nc.vector.tensor_scalar(out=pooled[:], in0=pooled[:],
                                    scalar1=convb_sb[:, 0:1], scalar2=None,
                                    op0=Alu.add)
            nc.vector.tensor_scalar(out=pooled[:], in0=pooled[:],
                                    scalar1=0.0, scalar2=None, op0=Alu.max)

            # ---- FC partial: red[oc, l] = sum_p pooled*fcw ----
            red = big.tile([OC, 3], f32, tag="red")
            fc_scr = big.tile([OC, PO_LOC], f32, tag="fcscr")
            for l in range(3):
                nc.vector.tensor_tensor(
                    out=fc_scr[:],
                    in0=pooled[:],
                    in1=fcw_sb[:, l * PO_LOC:(l + 1) * PO_LOC],
                    op=Alu.mult,
                )
                nc.vector.tensor_reduce(
                    out=red[:, l:l + 1], in_=fc_scr[:],
                    axis=mybir.AxisListType.X, op=Alu.add,
                )
            # sentiment branch (zeroed on h==1 cores)
            rs = sm.tile([128, 1], f32, tag="rs")
            nc.vector.tensor_scalar(out=rs[:], in0=sentv_sb[:], scalar1=0.0,
                                    scalar2=None, op0=Alu.max)
            tmp3 = sm.tile([128, 3], f32, tag="tmp3")
            nc.vector.tensor_scalar(out=tmp3[:], in0=sentw_sb[:],
                                    scalar1=rs[:, 0:1], scalar2=None, op0=Alu.mult)
            nc.vector.tensor_tensor(out=red[:], in0=red[:], in1=tmp3[:], op=Alu.add)

            psum_out = ps_o.tile([3, 1], f32, tag="po")
            nc.tensor.matmul(out=psum_out[:], lhsT=red[:], rhs=ones_sb[:],
                             start=True, stop=True)
            out_sb = sm.tile([3, 1], f32, tag="outsb")
            nc.vector.tensor_tensor(out=out_sb[:], in0=psum_out[:], in1=fcb_sb[:],
                                    op=Alu.add)
            nc.sync.dma_start(out=out, in_=out_sb[:])

    nc.compile()
    return nc


def _prep_core_inputs(core, input_ids, attention_mask, sentiment, logits,
                      hidden, conv_w, conv_b, fc_w, fc_b):
    b, h = core // 2, core % 2
    g0 = Y_LOC * h
    ext0 = g0 - 2

    lg = np.zeros((EXT, V), np.float32)
    idl = np.zeros((EXT, 1), np.int32)
    mk = np.zeros((EXT, 1), np.float32)
    hd = np.zeros((EXT, H), np.float32)
    lo = max(0, -ext0)            # local index where valid rows start
    s0, s1 = ext0 + lo, ext0 + EXT
    lg[lo:] = logits[b, s0:s1]
    idl[lo:, 0] = input_ids[b, s0:s1].astype(np.int32)
    mk[lo:, 0] = attention_mask[b, s0:s1]
    hd[lo:] = hidden[b, s0:s1]

    wt = np.ascontiguousarray(
        conv_w[:, :H, :].transpose(1, 2, 0).reshape(H, K * OC))
    ws = np.ascontiguousarray(conv_w[:, H, :].T)           # [K, OC]
    cb = np.ascontiguousarray(conv_b[:, None])             # [OC, 1]

    w3 = fc_w[:, :OC * 204].reshape(3, OC, 204)
    fcw = np.ascontiguousarray(
        w3[:, :, h * PO_LOC:(h + 1) * PO_LOC].transpose(1, 0, 2).reshape(OC, 3 * PO_LOC))

    sv = np.zeros((128, 1), np.float32)
    sw = np.zeros((128, 3), np.float32)
    fb = np.zeros((3, 1), np.float32)
    if h == 0:
        sv[:3, 0] = sentiment[b]
        sw[:3, :] = fc_w[:, OC * 204:].T                   # [3 j, 3 l]
        fb[:, 0] = fc_b

    return {
        "logits_loc": lg, "ids_loc": idl, "mask_loc": mk, "hidden_loc": hd,
        "wt": wt, "wsurp": ws, "convb": cb, "fcw": fcw,
        "sentv": sv, "sentw": sw, "fcb": fb,
    }


def _install_ntff_hook():
    import sys
    import types
    try:
        import antenv
        from trn_agent_boot.trn_boot import _ntff_profile_via_ctypes
    except ImportError:
        return
    if "antenv.axon_hooks" in sys.modules:
        return
    mod = types.ModuleType("antenv.axon_hooks")
    _h = [None]
    mod.set_axon_ntff_profile_hook = lambda hk: _h.__setitem__(0, hk)
    mod.get_axon_ntff_profile_hook = lambda: _h[0]
    sys.modules["antenv.axon_hooks"] = mod
    antenv.axon_hooks = mod
    try:
        mod.set_axon_ntff_profile_hook(
            _ntff_profile_via_ctypes('/opt/axon/libaxon_pjrt.so'))
    except Exception:
        pass


def kernel(input_ids, attention_mask, sentiment, logits, hidden,
           conv_w, conv_b, fc_w, fc_b, _trace=False):
    from concourse.bass_utils import run_bass_kernel_spmd

    input_ids = np.asarray(input_ids)
    attention_mask = np.asarray(attention_mask, np.float32)
    sentiment = np.asarray(sentiment, np.float32)
    logits = np.asarray(logits, np.float32)
    hidden = np.asarray(hidden, np.float32)
    conv_w = np.asarray(conv_w, np.float32)
    conv_b = np.asarray(conv_b, np.float32)
    fc_w = np.asarray(fc_w, np.float32)
    fc_b = np.asarray(fc_b, np.float32)

    if "nc" not in _CACHE:
        _CACHE["nc"] = _build_program()
    nc = _CACHE["nc"]

    in_maps = [
        _prep_core_inputs(c, input_ids, attention_mask, sentiment, logits,
                          hidden, conv_w, conv_b, fc_w, fc_b)
        for c in range(N_CORES)
    ]
    if _trace:
        _install_ntff_hook()
    res = run_bass_kernel_spmd(nc, in_maps, list(range(N_CORES)), trace=_trace)
    _CACHE["last_result"] = res

    out = np.zeros((B, 3), np.float32)
    for b in range(B):
        out[b] = (res.results[2 * b]["out_loc"][:, 0]
                  + res.results[2 * b + 1]["out_loc"][:, 0])
    return out



# revision 2
# speedup vs baseline: 1.6830x; 1.6830x over previous
"""Trainium2 Bass kernel for nn_CNN_80221399155117.

Pipeline: full-vocab softmax -> token-prob gather -> -log2 surprisal ->
concat(hidden, surp) -> Conv1d(k=5, pad=2) -> MaxPool1d(5) -> ReLU -> FC.

Sharding: 8 cores = (batch b, seq-half h). Each core owns the pool-aligned
conv-output range [510h, 510h+510) of its batch, needing feats rows
[510h-2, 510h+512) (EXT=514, zero-padded outside [0,1024)). The softmax
normalizer is computed locally per row (positions sharded, vocab local).

Perf structure:
- logits staged bf16 (halves HBM traffic); hidden host-transposed bf16.
- vocab chunks split between ScalarE (exact EXP + accum) and DVE
  (Schraudolph fast-exp: i16 = round(A*x + B) -> bitcast bf16 -> reduce;
  both ops run in the DVE 4x perf mode). The ~1.8% per-element error is
  bias-corrected in B and averages out in the 32000-term softmax sum.
- token-logit gather via indirect DMA (exact bf16 logit).
- conv as 80 accumulated matmuls vs resident transposed activations; the
  surprisal channel closes the accumulation with 5 rank-1 matmuls against
  the assembled surp row (no DMA round-trips in the epilogue).
"""

import numpy as np

B, S, V, H = 4, 1024, 32000, 2048
OC, K = 128, 5
N_CORES = 8
Y_LOC = 510            # conv output positions per core (102 pool windows)
PO_LOC = 102           # pooled cols per core
EXT = 514              # feats rows incl conv halo (510 + 2 + 2)
CF = 4000              # vocab chunk (free-dim) size
NCH = V // CF          # 8 chunks
NT = 4                 # main row tiles of 128
NHALO = EXT - 512      # 2 halo rows, packed [128, HF]
HQ = 128 // NHALO      # partitions per halo row
HF = V // HQ           # free elems per partition
LOG2E = 1.4426950408889634

A16 = 184.6650030622249        # 2^7 / ln 2
B16C = 16248.638470970125      # 127*2^7 + 0.5ulp-tuned bias correction
SCAL_CH = (0, 3, 6)            # chunks on ScalarE (exact exp)

_CACHE = {}


def _build_program():
    import concourse.tile as tile
    from concourse import bacc, bass, mybir
    from concourse.masks import make_identity

    f32 = mybir.dt.float32
    bf16 = mybir.dt.bfloat16
    i16 = mybir.dt.int16
    i32 = mybir.dt.int32
    Alu = mybir.AluOpType
    Act = mybir.ActivationFunctionType

    nc = bacc.Bacc("TRN2", target_bir_lowering=False, debug=False,
                   num_devices=N_CORES)

    lg = nc.dram_tensor("lg16", [EXT, V], bf16, kind="ExternalInput").ap()
    hidt = nc.dram_tensor("hidt", [128, 16 * EXT], bf16, kind="ExternalInput").ap()
    wt16 = nc.dram_tensor("wt16", [128, 16 * K * OC], bf16, kind="ExternalInput").ap()
    ids = nc.dram_tensor("ids", [EXT, 1], i32, kind="ExternalInput").ap()
    blob = nc.dram_tensor("blob", [128, BS], f32, kind="ExternalInput").ap()
    out = nc.dram_tensor("out_loc", [3, 1], f32, kind="ExternalOutput").ap()

    lg_flat = bass.AP(lg.tensor, 0, [[1, EXT * V], [1, 1]])

    with tile.TileContext(nc) as tc:
        with (
            tc.tile_pool(name="lp", bufs=8) as lp,          # logits chunks
            tc.tile_pool(name="eo", bufs=2) as eo,          # scalar exp out
            tc.tile_pool(name="si", bufs=3) as si,          # dve i16 bits
            tc.tile_pool(name="big", bufs=1) as big,        # resident
            tc.tile_pool(name="sm", bufs=10) as sm,         # small tiles
            tc.tile_pool(name="ps_t", bufs=2, space="PSUM") as ps_t,
            tc.tile_pool(name="ps_y", bufs=1, space="PSUM") as ps_y,
            tc.tile_pool(name="ps_o", bufs=2, space="PSUM") as ps_o,
        ):
            # ---- resident loads (ACT HWDGE ring; sync ring stays on logits) --
            xt = big.tile([128, 16 * EXT], bf16, tag="xt")
            nc.scalar.dma_start(out=xt[:], in_=hidt)
            wtile = big.tile([128, 16 * K * OC], bf16, tag="wtile")
            nc.scalar.dma_start(out=wtile[:], in_=wt16)
            blob_sb = big.tile([128, BS], f32, tag="blob")
            nc.scalar.dma_start(out=blob_sb[:], in_=blob)

            ident = big.tile([128, 128], f32, tag="ident")
            make_identity(nc, ident[:])
            ones_sb = big.tile([128, 1], f32, tag="ones")
            nc.vector.memset(ones_sb[:], 1.0)

            # stats [128, 10]: col 2t = sum(exp) tile t, col 2t+1 = gathered
            # logit; t=4 is the halo slab (partitions 0-1 live).
            stats = big.tile([128, 2 * (NT + 1)], f32, tag="stats")
            nc.vector.memset(stats[:], 1.0)   # ln(1)=0 on unused lanes
            gb = big.tile([128, NT + 1], bf16, tag="gb")
            nc.vector.memset(gb[:], 0.0)

            # ---- gather flat indices (gpsimd/SWDGE + tiny vector math) ----
            ids_all = sm.tile([128, NT], i32, tag="ids")
            nc.gpsimd.dma_start(out=ids_all[:],
                                in_=bass.AP(ids.tensor, 0, [[1, 128], [128, NT]]))
            iota_t = sm.tile([128, NT], i32, tag="iota")
            nc.gpsimd.iota(iota_t[:], pattern=[[1, NT]], base=0,
                           channel_multiplier=0)
            nc.vector.tensor_scalar(out=iota_t[:], in0=iota_t[:],
                                    scalar1=128 * V, scalar2=None, op0=Alu.mult)
            iota_p = sm.tile([128, 1], i32, tag="iotap")
            nc.gpsimd.iota(iota_p[:], pattern=[[1, 1]], base=0,
                           channel_multiplier=V)
            flat_all = sm.tile([128, NT], i32, tag="flat")
            nc.vector.tensor_tensor(out=flat_all[:], in0=ids_all[:],
                                    in1=iota_t[:], op=Alu.add)
            nc.vector.tensor_tensor(out=flat_all[:], in0=flat_all[:],
                                    in1=iota_p[:].to_broadcast([128, NT]),
                                    op=Alu.add)
            for t in range(NT):
                # HW DGE honors one index per partition per transfer
                nc.gpsimd.indirect_dma_start(
                    out=gb[:, t:t + 1], out_offset=None, in_=lg_flat,
                    in_offset=bass.IndirectOffsetOnAxis(
                        ap=flat_all[:, t:t + 1], axis=0))
            hrb = sm.tile([128, 1], i32, tag="hrb")
            nc.gpsimd.iota(hrb[:NHALO, :], pattern=[[1, 1]], base=512 * V,
                           channel_multiplier=V)
            hids = sm.tile([128, 1], i32, tag="hids")
            nc.gpsimd.dma_start(out=hids[:NHALO, :], in_=ids[512:EXT, :])
            hfl = sm.tile([128, 1], i32, tag="hfl")
            nc.vector.tensor_tensor(out=hfl[:NHALO, :], in0=hids[:NHALO, :],
                                    in1=hrb[:NHALO, :], op=Alu.add)
            nc.gpsimd.indirect_dma_start(
                out=gb[:NHALO, NT:], out_offset=None, in_=lg_flat,
                in_offset=bass.IndirectOffsetOnAxis(ap=hfl[:NHALO, :1], axis=0))

            # ---- conv: 80 matmuls accumulate into one PSUM bank ----
            psum_y = ps_y.tile([OC, Y_LOC], f32, tag="y")
            first = True
            for cc in range(16):
                for k in range(K):
                    nc.tensor.matmul(
                        out=psum_y[:],
                        lhsT=wtile[:, cc * 640 + k * 128: cc * 640 + (k + 1) * 128],
                        rhs=xt[:, cc * EXT + k: cc * EXT + k + Y_LOC],
                        start=first,
                        stop=False,
                    )
                    first = False

            # ---- main pass: per row tile, 8 vocab chunks split scalar/DVE --
            for t in range(NT):
                r0 = 128 * t
                sums = sm.tile([128, NCH], f32, tag=f"sums{t}")
                for ci in range(NCH):
                    x_sb = lp.tile([128, CF], bf16, tag="x")
                    nc.sync.dma_start(
                        out=x_sb[:],
                        in_=lg[r0:r0 + 128, ci * CF:(ci + 1) * CF],
                    )
                    if ci in SCAL_CH:
                        e_sb = eo.tile([128, CF], bf16, tag="e")
                        nc.scalar.activation(
                            out=e_sb[:], in_=x_sb[:], func=Act.Exp,
                            accum_out=sums[:, ci:ci + 1],
                        )
                    else:
                        q_sb = si.tile([128, CF], i16, tag="q")
                        nc.vector.tensor_scalar(
                            out=q_sb[:], in0=x_sb[:], scalar1=A16,
                            scalar2=B16C, op0=Alu.mult, op1=Alu.add)
                        nc.vector.tensor_reduce(
                            out=sums[:, ci:ci + 1], in_=q_sb[:].bitcast(bf16),
                            axis=mybir.AxisListType.X, op=Alu.add)
                nc.vector.tensor_reduce(
                    out=stats[:, 2 * t:2 * t + 1], in_=sums[:],
                    axis=mybir.AxisListType.X, op=Alu.add)

            # ---- halo rows (2): vocab packed across partitions ----
            hx = lp.tile([128, HF], bf16, tag="x")
            halo_src = bass.AP(lg.tensor, 512 * V,
                               [[V, NHALO], [HF, HQ], [1, HF]])
            nc.sync.dma_start(out=hx[:], in_=halo_src)
            he = eo.tile([128, HF], bf16, tag="e")
            hsums = sm.tile([128, 1], f32, tag="hsums")
            nc.scalar.activation(out=he[:], in_=hx[:], func=Act.Exp,
                                 accum_out=hsums[:])
            hsel = big.tile([128, NHALO], f32, tag="hsel")
            nc.vector.memset(hsel[:], 0.0)
            for a in range(NHALO):
                nc.vector.memset(hsel[a * HQ:(a + 1) * HQ, a:a + 1], 1.0)
            psum_h = ps_o.tile([NHALO, 1], f32, tag="ph")
            nc.tensor.matmul(out=psum_h[:], lhsT=hsel[:], rhs=hsums[:],
                             start=True, stop=True)
            nc.vector.tensor_copy(out=stats[:NHALO, 2 * NT:2 * NT + 1],
                                  in_=psum_h[:])

            # ---- gathered logits -> stats odd cols (f32) ----
            nc.vector.tensor_copy(out=stats[:, 1:2 * (NT + 1):2], in_=gb[:])

            # ---- batched surp: (ln(se) - g) * mask  [128, 5] ----
            lse_all = sm.tile([128, NT + 1], f32, tag="lse")
            nc.scalar.activation(out=lse_all[:], in_=stats[:, 0:2 * (NT + 1):2],
                                 func=Act.Ln)
            surp_all = sm.tile([128, NT + 1], f32, tag="surp")
            nc.vector.tensor_tensor(out=surp_all[:], in0=lse_all[:],
                                    in1=stats[:, 1:2 * (NT + 1):2],
                                    op=Alu.subtract)
            nc.vector.tensor_tensor(out=surp_all[:], in0=surp_all[:],
                                    in1=blob_sb[:, MASKC0:MASKC0 + NT + 1],
                                    op=Alu.mult)

            # ---- transpose surp to one row [1, EXT] via TensorE ----
            srow = big.tile([1, EXT], f32, tag="srow")
            for t in range(NT):
                tp = ps_t.tile([128, 128], f32, tag="tp")
                nc.tensor.transpose(out=tp[:1, :128], in_=surp_all[:, t:t + 1],
                                    identity=ident[:, :])
                nc.vector.tensor_copy(out=srow[0:1, 128 * t:128 * (t + 1)],
                                      in_=tp[0:1, :128])
            tp = ps_t.tile([128, 128], f32, tag="tp")
            nc.tensor.transpose(out=tp[:1, :NHALO],
                                in_=surp_all[:NHALO, NT:NT + 1],
                                identity=ident[:NHALO, :NHALO])
            nc.vector.tensor_copy(out=srow[0:1, 512:EXT], in_=tp[0:1, :NHALO])

            # ---- surp channel: 5 rank-1 matmuls close the accumulation ----
            for k in range(K):
                nc.tensor.matmul(
                    out=psum_y[:],
                    lhsT=blob_sb[0:1, WSURP0 + k * OC: WSURP0 + (k + 1) * OC],
                    rhs=srow[0:1, k:k + Y_LOC],
                    start=False,
                    stop=(k == K - 1),
                )

            # ---- maxpool(5) + bias + relu ----
            pooled = big.tile([OC, PO_LOC], f32, tag="pooled")
            stop_off = K * (PO_LOC - 1) + 1
            nc.vector.tensor_copy(out=pooled[:], in_=psum_y[:, 0:stop_off:K])
            for j in range(1, K):
                nc.vector.tensor_tensor(out=pooled[:], in0=pooled[:],
                                        in1=psum_y[:, j:j + stop_off:K],
                                        op=Alu.max)
            nc.vector.tensor_scalar(out=pooled[:], in0=pooled[:],
                                    scalar1=blob_sb[:, CONVB0:CONVB0 + 1],
                                    scalar2=0.0, op0=Alu.add, op1=Alu.max)

            # ---- FC partial: red[oc, l] = sum_j pooled*fcw ----
            red = big.tile([OC, 3], f32, tag="red")
            fc_scr = big.tile([OC, PO_LOC], f32, tag="fcscr")
            for l in range(3):
                nc.vector.tensor_tensor(
                    out=fc_scr[:],
                    in0=pooled[:],
                    in1=blob_sb[:, FCW0 + l * PO_LOC: FCW0 + (l + 1) * PO_LOC],
                    op=Alu.mult,
                )
                nc.vector.tensor_reduce(
                    out=red[:, l:l + 1], in_=fc_scr[:],
                    axis=mybir.AxisListType.X, op=Alu.add,
                )
            # sentiment branch (zeroed on h==1 cores)
            rs = sm.tile([128, 1], f32, tag="rs")
            nc.vector.tensor_scalar(out=rs[:], in0=blob_sb[:, SENTV0:SENTV0 + 1],
                                    scalar1=0.0, scalar2=None, op0=Alu.max)
            tmp3 = sm.tile([128, 3], f32, tag="tmp3")
            nc.vector.tensor_scalar(out=tmp3[:],
                                    in0=blob_sb[:, SENTW0:SENTW0 + 3],
                                    scalar1=rs[:, 0:1], scalar2=None,
                                    op0=Alu.mult)
            nc.vector.tensor_tensor(out=red[:], in0=red[:], in1=tmp3[:],
                                    op=Alu.add)

            psum_out = ps_o.tile([3, 1], f32, tag="po")
            nc.tensor.matmul(out=psum_out[:], lhsT=red[:], rhs=ones_sb[:],
                             start=True, stop=True)
            out_sb = sm.tile([3, 1], f32, tag="outsb")
            nc.vector.tensor_tensor(out=out_sb[:], in0=psum_out[:],
                                    in1=blob_sb[0:3, FCB0:FCB0 + 1],
                                    op=Alu.add)
            nc.sync.dma_start(out=out, in_=out_sb[:])

    nc.compile()
    return nc


# blob column layout (f32 [128, BS])
MASKC0 = 0                      # [128, NT+1] attention mask, col-major tiles
WSURP0 = MASKC0 + NT + 1        # [1, K*OC] surp conv weights * LOG2E (part 0)
CONVB0 = WSURP0 + K * OC        # [OC, 1] conv bias
FCW0 = CONVB0 + 1               # [OC, 3*PO_LOC] fc weights for this shard
SENTV0 = FCW0 + 3 * PO_LOC      # [3, 1] sentiment values (h==0 only)
SENTW0 = SENTV0 + 1             # [3, 3] fc weights for sentiment cols
FCB0 = SENTW0 + 3               # [3, 1] fc bias (h==0 only)
BS = FCB0 + 1


def _prep_core_inputs(core, input_ids, attention_mask, sentiment, logits,
                      hidden, conv_w, conv_b, fc_w, fc_b, bf16):
    b, h = core // 2, core % 2
    g0 = Y_LOC * h
    ext0 = g0 - 2
    lo = max(0, -ext0)            # local index where valid rows start
    s0, s1 = ext0 + lo, ext0 + EXT

    lg = np.zeros((EXT, V), bf16)
    lg[lo:] = logits[b, s0:s1].astype(bf16)
    idl = np.zeros((EXT, 1), np.int32)
    idl[lo:, 0] = input_ids[b, s0:s1].astype(np.int32)

    mask_ext = np.zeros((EXT,), np.float32)
    mask_ext[lo:] = attention_mask[b, s0:s1]

    # hidden, transposed to [128, 16*EXT]: col cc*EXT+j = hidden[s, cc*128+p]
    hd = np.zeros((EXT, H), np.float32)
    hd[lo:] = hidden[b, s0:s1]
    hidt = np.ascontiguousarray(
        hd.T.astype(bf16).reshape(16, 128, EXT).transpose(1, 0, 2)
        .reshape(128, 16 * EXT))

    # conv weights [128, 16*640]: wt16[p, cc*640 + k*128 + oc] = w[oc, cc*128+p, k]
    wt = conv_w[:, :H, :].transpose(1, 2, 0).reshape(16, 128, K * OC)
    wt16 = np.ascontiguousarray(wt.transpose(1, 0, 2).reshape(128, 16 * K * OC)
                                .astype(bf16))

    blob = np.zeros((128, BS), np.float32)
    mcol = np.zeros((128, NT + 1), np.float32)
    mcol[:, :NT] = mask_ext[:512].reshape(NT, 128).T
    mcol[:NHALO, NT] = mask_ext[512:]
    blob[:, MASKC0:MASKC0 + NT + 1] = mcol
    blob[0, WSURP0:WSURP0 + K * OC] = (conv_w[:, H, :].T * LOG2E).reshape(-1)
    blob[:, CONVB0] = conv_b
    w3 = fc_w[:, :OC * 204].reshape(3, OC, 204)
    blob[:, FCW0:FCW0 + 3 * PO_LOC] = np.ascontiguousarray(
        w3[:, :, h * PO_LOC:(h + 1) * PO_LOC].transpose(1, 0, 2)
        .reshape(OC, 3 * PO_LOC))
    if h == 0:
        blob[:3, SENTV0] = sentiment[b]
        blob[:3, SENTW0:SENTW0 + 3] = fc_w[:, OC * 204:].T
        blob[:3, FCB0] = fc_b

    return {"lg16": lg, "hidt": hidt, "wt16": wt16, "ids": idl, "blob": blob}


def _install_ntff_hook():
    import sys
    import types
    try:
        import antenv
        from trn_agent_boot.trn_boot import _ntff_profile_via_ctypes
    except ImportError:
        return
    if "antenv.axon_hooks" in sys.modules:
        return
    mod = types.ModuleType("antenv.axon_hooks")
    _h = [None]
    mod.set_axon_ntff_profile_hook = lambda hk: _h.__setitem__(0, hk)
    mod.get_axon_ntff_profile_hook = lambda: _h[0]
    sys.modules["antenv.axon_hooks"] = mod
    antenv.axon_hooks = mod
    try:
        mod.set_axon_ntff_profile_hook(
            _ntff_profile_via_ctypes('/opt/axon/libaxon_pjrt.so'))
    except Exception:
        pass


def kernel(input_ids, attention_mask, sentiment, logits, hidden,
           conv_w, conv_b, fc_w, fc_b, _trace=False):
    import ml_dtypes
    from concourse.bass_utils import run_bass_kernel_spmd

    bf16 = ml_dtypes.bfloat16
    input_ids = np.asarray(input_ids)
    attention_mask = np.asarray(attention_mask, np.float32)
    sentiment = np.asarray(sentiment, np.float32)
    logits = np.asarray(logits, np.float32)
    hidden = np.asarray(hidden, np.float32)
    conv_w = np.asarray(conv_w, np.float32)
    conv_b = np.asarray(conv_b, np.float32)
    fc_w = np.asarray(fc_w, np.float32)
    fc_b = np.asarray(fc_b, np.float32)

    if "nc" not in _CACHE:
        _CACHE["nc"] = _build_program()
    nc = _CACHE["nc"]

    in_maps = [
        _prep_core_inputs(c, input_ids, attention_mask, sentiment, logits,
                          hidden, conv_w, conv_b, fc_w, fc_b, bf16)
        for c in range(N_CORES)
    ]
    if _trace:
        _install_ntff_hook()
    res = run_bass_kernel_spmd(nc, in_maps, list(range(N_CORES)), trace=_trace)
    _CACHE["last_result"] = res

    out = np.zeros((B, 3), np.float32)
    for b in range(B):
        out[b] = (res.results[2 * b]["out_loc"][:, 0]
                  + res.results[2 * b + 1]["out_loc"][:, 0])
    return out


# revision 3
# speedup vs baseline: 1.8180x; 1.0802x over previous
"""Trainium2 Bass kernel for nn_CNN_80221399155117.

Pipeline: full-vocab softmax -> token-prob gather -> -log2 surprisal ->
concat(hidden, surp) -> Conv1d(k=5, pad=2) -> MaxPool1d(5) -> ReLU -> FC.

Sharding: 8 cores = (batch b, seq-half h). Each core owns the pool-aligned
conv-output range [510h, 510h+510) of its batch, needing feats rows
[510h-2, 510h+512) (EXT=514, zero-padded outside [0,1024)). The softmax
normalizer is computed locally per row (positions sharded, vocab local).

Perf structure:
- logits staged bf16 (halves HBM traffic); hidden host-transposed bf16.
- vocab chunks split between ScalarE (exact EXP + accum) and DVE
  (Schraudolph fast-exp: i16 = round(A*x + B) -> bitcast bf16 -> reduce;
  both ops run in the DVE 4x perf mode). The ~1.8% per-element error is
  bias-corrected in B and averages out in the 32000-term softmax sum.
- token-logit gather via indirect DMA (exact bf16 logit).
- conv as 80 accumulated matmuls vs resident transposed activations; the
  surprisal channel closes the accumulation with 5 rank-1 matmuls against
  the assembled surp row (no DMA round-trips in the epilogue).
"""

import numpy as np

B, S, V, H = 4, 1024, 32000, 2048
OC, K = 128, 5
N_CORES = 8
Y_LOC = 510            # conv output positions per core (102 pool windows)
PO_LOC = 102           # pooled cols per core
EXT = 514              # feats rows incl conv halo (510 + 2 + 2)
CF = 4000              # vocab chunk (free-dim) size
NCH = V // CF          # 8 chunks
NT = 4                 # main row tiles of 128
NHALO = EXT - 512      # 2 halo rows, packed [128, HF]
HQ = 128 // NHALO      # partitions per halo row
HF = V // HQ           # free elems per partition
LOG2E = 1.4426950408889634

A16 = 184.6650030622249        # 2^7 / ln 2
B16C = 16248.638470970125      # 127*2^7 + 0.5ulp-tuned bias correction
SCAL_CH = (0, 1, 3, 5, 6)      # chunks on ScalarE (exact exp)

_CACHE = {}


def _build_program():
    import concourse.tile as tile
    from concourse import bacc, bass, mybir
    from concourse.masks import make_identity

    f32 = mybir.dt.float32
    bf16 = mybir.dt.bfloat16
    i16 = mybir.dt.int16
    i32 = mybir.dt.int32
    Alu = mybir.AluOpType
    Act = mybir.ActivationFunctionType

    nc = bacc.Bacc("TRN2", target_bir_lowering=False, debug=False,
                   num_devices=N_CORES)

    lg = nc.dram_tensor("lg16", [EXT, V], bf16, kind="ExternalInput").ap()
    hidt = nc.dram_tensor("hidt", [128, 16 * EXT], bf16, kind="ExternalInput").ap()
    wt16 = nc.dram_tensor("wt16", [128, 16 * K * OC], bf16, kind="ExternalInput").ap()
    ids = nc.dram_tensor("ids", [EXT, 1], i32, kind="ExternalInput").ap()
    blob = nc.dram_tensor("blob", [128, BS], f32, kind="ExternalInput").ap()
    out = nc.dram_tensor("out_loc", [3, 1], f32, kind="ExternalOutput").ap()

    lg_flat = bass.AP(lg.tensor, 0, [[1, EXT * V], [1, 1]])

    with tile.TileContext(nc) as tc:
        with (
            tc.tile_pool(name="lp", bufs=8) as lp,          # logits chunks
            tc.tile_pool(name="eo", bufs=2) as eo,          # scalar exp out
            tc.tile_pool(name="si", bufs=3) as si,          # dve i16 bits
            tc.tile_pool(name="big", bufs=1) as big,        # resident
            tc.tile_pool(name="sm", bufs=10) as sm,         # small tiles
            tc.tile_pool(name="ps_t", bufs=2, space="PSUM") as ps_t,
            tc.tile_pool(name="ps_y", bufs=1, space="PSUM") as ps_y,
            tc.tile_pool(name="ps_o", bufs=2, space="PSUM") as ps_o,
        ):
            # ---- resident loads (ACT HWDGE ring; sync ring stays on logits) --
            xt = big.tile([128, 16 * EXT], bf16, tag="xt")
            nc.scalar.dma_start(out=xt[:], in_=hidt)
            wtile = big.tile([128, 16 * K * OC], bf16, tag="wtile")
            nc.scalar.dma_start(out=wtile[:], in_=wt16)
            blob_sb = big.tile([128, BS], f32, tag="blob")
            nc.scalar.dma_start(out=blob_sb[:], in_=blob)

            ident = big.tile([128, 128], f32, tag="ident")
            make_identity(nc, ident[:])
            ones_sb = big.tile([128, 1], f32, tag="ones")
            nc.vector.memset(ones_sb[:], 1.0)

            # stats [128, 10]: col 2t = sum(exp) tile t, col 2t+1 = gathered
            # logit; t=4 is the halo slab (partitions 0-1 live).
            stats = big.tile([128, 2 * (NT + 1)], f32, tag="stats")
            nc.vector.memset(stats[:], 1.0)   # ln(1)=0 on unused lanes
            gb = big.tile([128, NT + 1], bf16, tag="gb")
            nc.vector.memset(gb[:], 0.0)

            # ---- gather flat indices (gpsimd/SWDGE + tiny vector math) ----
            ids_all = sm.tile([128, NT], i32, tag="ids")
            nc.gpsimd.dma_start(out=ids_all[:],
                                in_=bass.AP(ids.tensor, 0, [[1, 128], [128, NT]]))
            iota_t = sm.tile([128, NT], i32, tag="iota")
            nc.gpsimd.iota(iota_t[:], pattern=[[1, NT]], base=0,
                           channel_multiplier=0)
            nc.vector.tensor_scalar(out=iota_t[:], in0=iota_t[:],
                                    scalar1=128 * V, scalar2=None, op0=Alu.mult)
            iota_p = sm.tile([128, 1], i32, tag="iotap")
            nc.gpsimd.iota(iota_p[:], pattern=[[1, 1]], base=0,
                           channel_multiplier=V)
            flat_all = sm.tile([128, NT], i32, tag="flat")
            nc.vector.tensor_tensor(out=flat_all[:], in0=ids_all[:],
                                    in1=iota_t[:], op=Alu.add)
            nc.vector.tensor_tensor(out=flat_all[:], in0=flat_all[:],
                                    in1=iota_p[:].to_broadcast([128, NT]),
                                    op=Alu.add)
            for t in range(NT):
                # HW DGE honors one index per partition per transfer
                nc.gpsimd.indirect_dma_start(
                    out=gb[:, t:t + 1], out_offset=None, in_=lg_flat,
                    in_offset=bass.IndirectOffsetOnAxis(
                        ap=flat_all[:, t:t + 1], axis=0))
            hrb = sm.tile([128, 1], i32, tag="hrb")
            nc.gpsimd.iota(hrb[:NHALO, :], pattern=[[1, 1]], base=512 * V,
                           channel_multiplier=V)
            hids = sm.tile([128, 1], i32, tag="hids")
            nc.gpsimd.dma_start(out=hids[:NHALO, :], in_=ids[512:EXT, :])
            hfl = sm.tile([128, 1], i32, tag="hfl")
            nc.vector.tensor_tensor(out=hfl[:NHALO, :], in0=hids[:NHALO, :],
                                    in1=hrb[:NHALO, :], op=Alu.add)
            nc.gpsimd.indirect_dma_start(
                out=gb[:NHALO, NT:], out_offset=None, in_=lg_flat,
                in_offset=bass.IndirectOffsetOnAxis(ap=hfl[:NHALO, :1], axis=0))

            # ---- conv: 80 matmuls accumulate into one PSUM bank ----
            psum_y = ps_y.tile([OC, Y_LOC], f32, tag="y")
            first = True
            for cc in range(16):
                for k in range(K):
                    nc.tensor.matmul(
                        out=psum_y[:],
                        lhsT=wtile[:, cc * 640 + k * 128: cc * 640 + (k + 1) * 128],
                        rhs=xt[:, cc * EXT + k: cc * EXT + k + Y_LOC],
                        start=first,
                        stop=False,
                    )
                    first = False

            # ---- main pass: per row tile, 8 vocab chunks split scalar/DVE --
            for t in range(NT):
                r0 = 128 * t
                sums = sm.tile([128, NCH], f32, tag=f"sums{t}")
                for ci in range(NCH):
                    x_sb = lp.tile([128, CF], bf16, tag="x")
                    nc.sync.dma_start(
                        out=x_sb[:],
                        in_=lg[r0:r0 + 128, ci * CF:(ci + 1) * CF],
                    )
                    if ci in SCAL_CH:
                        e_sb = eo.tile([128, CF], bf16, tag="e")
                        nc.scalar.activation(
                            out=e_sb[:], in_=x_sb[:], func=Act.Exp,
                            accum_out=sums[:, ci:ci + 1],
                        )
                    else:
                        q_sb = si.tile([128, CF], i16, tag="q")
                        nc.vector.tensor_scalar(
                            out=q_sb[:], in0=x_sb[:], scalar1=A16,
                            scalar2=B16C, op0=Alu.mult, op1=Alu.add)
                        nc.vector.tensor_reduce(
                            out=sums[:, ci:ci + 1], in_=q_sb[:].bitcast(bf16),
                            axis=mybir.AxisListType.X, op=Alu.add)
                nc.vector.tensor_reduce(
                    out=stats[:, 2 * t:2 * t + 1], in_=sums[:],
                    axis=mybir.AxisListType.X, op=Alu.add)

            # ---- halo rows (2): vocab packed across partitions ----
            hx = lp.tile([128, HF], bf16, tag="x")
            halo_src = bass.AP(lg.tensor, 512 * V,
                               [[V, NHALO], [HF, HQ], [1, HF]])
            nc.sync.dma_start(out=hx[:], in_=halo_src)
            he = eo.tile([128, HF], bf16, tag="e")
            hsums = sm.tile([128, 1], f32, tag="hsums")
            nc.scalar.activation(out=he[:], in_=hx[:], func=Act.Exp,
                                 accum_out=hsums[:])
            hsel = big.tile([128, NHALO], f32, tag="hsel")
            nc.vector.memset(hsel[:], 0.0)
            for a in range(NHALO):
                nc.vector.memset(hsel[a * HQ:(a + 1) * HQ, a:a + 1], 1.0)
            psum_h = ps_o.tile([NHALO, 1], f32, tag="ph")
            nc.tensor.matmul(out=psum_h[:], lhsT=hsel[:], rhs=hsums[:],
                             start=True, stop=True)
            nc.vector.tensor_copy(out=stats[:NHALO, 2 * NT:2 * NT + 1],
                                  in_=psum_h[:])

            # ---- gathered logits -> stats odd cols (f32) ----
            nc.vector.tensor_copy(out=stats[:, 1:2 * (NT + 1):2], in_=gb[:])

            # ---- batched surp: (ln(se) - g) * mask  [128, 5] ----
            lse_all = sm.tile([128, NT + 1], f32, tag="lse")
            nc.scalar.activation(out=lse_all[:], in_=stats[:, 0:2 * (NT + 1):2],
                                 func=Act.Ln)
            surp_all = sm.tile([128, NT + 1], f32, tag="surp")
            nc.vector.tensor_tensor(out=surp_all[:], in0=lse_all[:],
                                    in1=stats[:, 1:2 * (NT + 1):2],
                                    op=Alu.subtract)
            nc.vector.tensor_tensor(out=surp_all[:], in0=surp_all[:],
                                    in1=blob_sb[:, MASKC0:MASKC0 + NT + 1],
                                    op=Alu.mult)

            # ---- transpose surp to one row [1, EXT] via TensorE ----
            srow = big.tile([1, EXT], f32, tag="srow")
            for t in range(NT):
                tp = ps_t.tile([128, 128], f32, tag="tp")
                nc.tensor.transpose(out=tp[:1, :128], in_=surp_all[:, t:t + 1],
                                    identity=ident[:, :])
                nc.vector.tensor_copy(out=srow[0:1, 128 * t:128 * (t + 1)],
                                      in_=tp[0:1, :128])
            tp = ps_t.tile([128, 128], f32, tag="tp")
            nc.tensor.transpose(out=tp[:1, :NHALO],
                                in_=surp_all[:NHALO, NT:NT + 1],
                                identity=ident[:NHALO, :NHALO])
            nc.vector.tensor_copy(out=srow[0:1, 512:EXT], in_=tp[0:1, :NHALO])

            # ---- surp channel: 5 rank-1 matmuls close the accumulation ----
            for k in range(K):
                nc.tensor.matmul(
                    out=psum_y[:],
                    lhsT=blob_sb[0:1, WSURP0 + k * OC: WSURP0 + (k + 1) * OC],
                    rhs=srow[0:1, k:k + Y_LOC],
                    start=False,
                    stop=(k == K - 1),
                )

            # ---- maxpool(5) + bias + relu ----
            pooled = big.tile([OC, PO_LOC], f32, tag="pooled")
            stop_off = K * (PO_LOC - 1) + 1
            nc.vector.tensor_copy(out=pooled[:], in_=psum_y[:, 0:stop_off:K])
            for j in range(1, K):
                nc.vector.tensor_tensor(out=pooled[:], in0=pooled[:],
                                        in1=psum_y[:, j:j + stop_off:K],
                                        op=Alu.max)
            nc.vector.tensor_scalar(out=pooled[:], in0=pooled[:],
                                    scalar1=blob_sb[:, CONVB0:CONVB0 + 1],
                                    scalar2=0.0, op0=Alu.add, op1=Alu.max)

            # ---- FC partial: red[oc, l] = sum_j pooled*fcw ----
            red = big.tile([OC, 3], f32, tag="red")
            fc_scr = big.tile([OC, PO_LOC], f32, tag="fcscr")
            for l in range(3):
                nc.vector.tensor_tensor(
                    out=fc_scr[:],
                    in0=pooled[:],
                    in1=blob_sb[:, FCW0 + l * PO_LOC: FCW0 + (l + 1) * PO_LOC],
                    op=Alu.mult,
                )
                nc.vector.tensor_reduce(
                    out=red[:, l:l + 1], in_=fc_scr[:],
                    axis=mybir.AxisListType.X, op=Alu.add,
                )
            # sentiment branch (zeroed on h==1 cores)
            rs = sm.tile([128, 1], f32, tag="rs")
            nc.vector.tensor_scalar(out=rs[:], in0=blob_sb[:, SENTV0:SENTV0 + 1],
                                    scalar1=0.0, scalar2=None, op0=Alu.max)
            tmp3 = sm.tile([128, 3], f32, tag="tmp3")
            nc.vector.tensor_scalar(out=tmp3[:],
                                    in0=blob_sb[:, SENTW0:SENTW0 + 3],
                                    scalar1=rs[:, 0:1], scalar2=None,
                                    op0=Alu.mult)
            nc.vector.tensor_tensor(out=red[:], in0=red[:], in1=tmp3[:],
                                    op=Alu.add)

            psum_out = ps_o.tile([3, 1], f32, tag="po")
            nc.tensor.matmul(out=psum_out[:], lhsT=red[:], rhs=ones_sb[:],
                             start=True, stop=True)
            out_sb = sm.tile([3, 1], f32, tag="outsb")
            nc.vector.tensor_tensor(out=out_sb[:], in0=psum_out[:],
                                    in1=blob_sb[0:3, FCB0:FCB0 + 1],
                                    op=Alu.add)
            nc.sync.dma_start(out=out, in_=out_sb[:])

    nc.compile()
    return nc


# blob column layout (f32 [128, BS])
MASKC0 = 0                      # [128, NT+1] attention mask, col-major tiles
WSURP0 = MASKC0 + NT + 1        # [1, K*OC] surp conv weights * LOG2E (part 0)
CONVB0 = WSURP0 + K * OC        # [OC, 1] conv bias
FCW0 = CONVB0 + 1               # [OC, 3*PO_LOC] fc weights for this shard
SENTV0 = FCW0 + 3 * PO_LOC      # [3, 1] sentiment values (h==0 only)
SENTW0 = SENTV0 + 1             # [3, 3] fc weights for sentiment cols
FCB0 = SENTW0 + 3               # [3, 1] fc bias (h==0 only)
BS = FCB0 + 1


def _prep_core_inputs(core, input_ids, attention_mask, sentiment, logits,
                      hidden, conv_w, conv_b, fc_w, fc_b, bf16):
    b, h = core // 2, core % 2
    g0 = Y_LOC * h
    ext0 = g0 - 2
    lo = max(0, -ext0)            # local index where valid rows start
    s0, s1 = ext0 + lo, ext0 + EXT

    lg = np.zeros((EXT, V), bf16)
    lg[lo:] = logits[b, s0:s1].astype(bf16)
    idl = np.zeros((EXT, 1), np.int32)
    idl[lo:, 0] = input_ids[b, s0:s1].astype(np.int32)

    mask_ext = np.zeros((EXT,), np.float32)
    mask_ext[lo:] = attention_mask[b, s0:s1]

    # hidden, transposed to [128, 16*EXT]: col cc*EXT+j = hidden[s, cc*128+p]
    hd = np.zeros((EXT, H), np.float32)
    hd[lo:] = hidden[b, s0:s1]
    hidt = np.ascontiguousarray(
        hd.T.astype(bf16).reshape(16, 128, EXT).transpose(1, 0, 2)
        .reshape(128, 16 * EXT))

    # conv weights [128, 16*640]: wt16[p, cc*640 + k*128 + oc] = w[oc, cc*128+p, k]
    wt = conv_w[:, :H, :].transpose(1, 2, 0).reshape(16, 128, K * OC)
    wt16 = np.ascontiguousarray(wt.transpose(1, 0, 2).reshape(128, 16 * K * OC)
                                .astype(bf16))

    blob = np.zeros((128, BS), np.float32)
    mcol = np.zeros((128, NT + 1), np.float32)
    mcol[:, :NT] = mask_ext[:512].reshape(NT, 128).T
    mcol[:NHALO, NT] = mask_ext[512:]
    blob[:, MASKC0:MASKC0 + NT + 1] = mcol
    blob[0, WSURP0:WSURP0 + K * OC] = (conv_w[:, H, :].T * LOG2E).reshape(-1)
    blob[:, CONVB0] = conv_b
    w3 = fc_w[:, :OC * 204].reshape(3, OC, 204)
    blob[:, FCW0:FCW0 + 3 * PO_LOC] = np.ascontiguousarray(
        w3[:, :, h * PO_LOC:(h + 1) * PO_LOC].transpose(1, 0, 2)
        .reshape(OC, 3 * PO_LOC))
    if h == 0:
        blob[:3, SENTV0] = sentiment[b]
        blob[:3, SENTW0:SENTW0 + 3] = fc_w[:, OC * 204:].T
        blob[:3, FCB0] = fc_b

    return {"lg16": lg, "hidt": hidt, "wt16": wt16, "ids": idl, "blob": blob}


def _install_ntff_hook():
    import sys
    import types
    try:
        import antenv
        from trn_agent_boot.trn_boot import _ntff_profile_via_ctypes
    except ImportError:
        return
    if "antenv.axon_hooks" in sys.modules:
        return
    mod = types.ModuleType("antenv.axon_hooks")
    _h = [None]
    mod.set_axon_ntff_profile_hook = lambda hk: _h.__setitem__(0, hk)
    mod.get_axon_ntff_profile_hook = lambda: _h[0]
    sys.modules["antenv.axon_hooks"] = mod
    antenv.axon_hooks = mod
    try:
        mod.set_axon_ntff_profile_hook(
            _ntff_profile_via_ctypes('/opt/axon/libaxon_pjrt.so'))
    except Exception:
        pass


def kernel(input_ids, attention_mask, sentiment, logits, hidden,
           conv_w, conv_b, fc_w, fc_b, _trace=False):
    import ml_dtypes
    from concourse.bass_utils import run_bass_kernel_spmd

    bf16 = ml_dtypes.bfloat16
    input_ids = np.asarray(input_ids)
    attention_mask = np.asarray(attention_mask, np.float32)
    sentiment = np.asarray(sentiment, np.float32)
    logits = np.asarray(logits, np.float32)
    hidden = np.asarray(hidden, np.float32)
    conv_w = np.asarray(conv_w, np.float32)
    conv_b = np.asarray(conv_b, np.float32)
    fc_w = np.asarray(fc_w, np.float32)
    fc_b = np.asarray(fc_b, np.float32)

    if "nc" not in _CACHE:
        _CACHE["nc"] = _build_program()
    nc = _CACHE["nc"]

    in_maps = [
        _prep_core_inputs(c, input_ids, attention_mask, sentiment, logits,
                          hidden, conv_w, conv_b, fc_w, fc_b, bf16)
        for c in range(N_CORES)
    ]
    if _trace:
        _install_ntff_hook()
    res = run_bass_kernel_spmd(nc, in_maps, list(range(N_CORES)), trace=_trace)
    _CACHE["last_result"] = res

    out = np.zeros((B, 3), np.float32)
    for b in range(B):
        out[b] = (res.results[2 * b]["out_loc"][:, 0]
                  + res.results[2 * b + 1]["out_loc"][:, 0])
    return out


# revision 7
# speedup vs baseline: 1.8726x; 1.0300x over previous
"""Trainium2 Bass kernel for nn_CNN_80221399155117.

Pipeline: full-vocab softmax -> token-prob gather -> -log2 surprisal ->
concat(hidden, surp) -> Conv1d(k=5, pad=2) -> MaxPool1d(5) -> ReLU -> FC.

Sharding: 8 cores = (batch b, seq-half h). Each core owns the pool-aligned
conv-output range [510h, 510h+510) of its batch, needing feats rows
[510h-2, 510h+512) (EXT=514, zero-padded outside [0,1024)). The softmax
normalizer is computed locally per row (positions sharded, vocab local).

Perf structure:
- logits staged bf16 (halves HBM traffic); hidden host-transposed bf16.
- vocab chunks split between ScalarE (exact EXP + accum) and DVE
  (Schraudolph fast-exp: i16 = round(A*x + B) -> bitcast bf16 -> reduce;
  both ops run in the DVE 4x perf mode). The ~1.8% per-element error is
  bias-corrected in B and averages out in the 32000-term softmax sum.
- token-logit gather via indirect DMA (exact bf16 logit).
- conv as 80 accumulated matmuls vs resident transposed activations; the
  surprisal channel closes the accumulation with 5 rank-1 matmuls against
  the assembled surp row (no DMA round-trips in the epilogue).
"""

import numpy as np

B, S, V, H = 4, 1024, 32000, 2048
OC, K = 128, 5
N_CORES = 8
Y_LOC = 510            # conv output positions per core (102 pool windows)
PO_LOC = 102           # pooled cols per core
EXT = 514              # feats rows incl conv halo (510 + 2 + 2)
CF = 4000              # vocab chunk (free-dim) size
NCH = V // CF          # 8 chunks
NT = 4                 # main row tiles of 128
NHALO = EXT - 512      # 2 halo rows, packed [128, HF]
HQ = 128 // NHALO      # partitions per halo row
HF = V // HQ           # free elems per partition
LOG2E = 1.4426950408889634

A16 = 184.6650030622249        # 2^7 / ln 2
B16C = 16248.638470970125      # 127*2^7 + 0.5ulp-tuned bias correction
SCAL_CH = (0, 1, 3, 5, 6)      # chunks on ScalarE (exact exp)

_CACHE = {}


def _build_program():
    import concourse.tile as tile
    from concourse import bacc, bass, mybir
    from concourse.masks import make_identity

    f32 = mybir.dt.float32
    bf16 = mybir.dt.bfloat16
    i16 = mybir.dt.int16
    i32 = mybir.dt.int32
    Alu = mybir.AluOpType
    Act = mybir.ActivationFunctionType

    nc = bacc.Bacc("TRN2", target_bir_lowering=False, debug=False,
                   num_devices=N_CORES)

    lg = nc.dram_tensor("lg16", [EXT, V], bf16, kind="ExternalInput").ap()
    hidt = nc.dram_tensor("hidt", [128, 16 * EXT], bf16, kind="ExternalInput").ap()
    wt16 = nc.dram_tensor("wt16", [128, 17 * K * OC], bf16,
                          kind="ExternalInput").ap()
    ids = nc.dram_tensor("ids", [128 * (NT + 1), 1], i32,
                         kind="ExternalInput").ap()
    blob = nc.dram_tensor("blob", [128, BS], f32, kind="ExternalInput").ap()
    out = nc.dram_tensor("out_loc", [3, 1], f32, kind="ExternalOutput").ap()

    lg_flat = bass.AP(lg.tensor, 0, [[1, EXT * V], [1, 1]])

    with tile.TileContext(nc) as tc:
        with (
            tc.tile_pool(name="lp", bufs=8) as lp,          # logits chunks
            tc.tile_pool(name="eo", bufs=2) as eo,          # scalar exp out
            tc.tile_pool(name="si", bufs=3) as si,          # dve i16 bits
            tc.tile_pool(name="big", bufs=1) as big,        # resident
            tc.tile_pool(name="sm", bufs=10) as sm,         # small tiles
            tc.tile_pool(name="ps_t", bufs=2, space="PSUM") as ps_t,
            tc.tile_pool(name="ps_y", bufs=1, space="PSUM") as ps_y,
            tc.tile_pool(name="ps_o", bufs=2, space="PSUM") as ps_o,
        ):
            # ---- resident loads (ACT HWDGE ring; sync ring stays on logits) --
            xt = big.tile([128, 16 * EXT], bf16, tag="xt")
            nc.scalar.dma_start(out=xt[:], in_=hidt)
            wtile = big.tile([128, 17 * K * OC], bf16, tag="wtile")
            nc.scalar.dma_start(out=wtile[:], in_=wt16)
            blob_sb = big.tile([128, BS], f32, tag="blob")
            nc.scalar.dma_start(out=blob_sb[:], in_=blob)
            ids_all = sm.tile([128, NT + 1], i32, tag="ids")
            nc.scalar.dma_start(
                out=ids_all[:],
                in_=bass.AP(ids.tensor, 0, [[1, 128], [128, NT + 1]]))

            identb = big.tile([128, 128], bf16, tag="identb")
            make_identity(nc, identb[:])
            ones_sb = big.tile([128, 1], f32, tag="ones")
            nc.vector.memset(ones_sb[:], 1.0)

            # stats [128, 10]: col 2t = sum(exp) tile t, col 2t+1 = gathered
            # logit; t=4 is the halo slab (partitions 0-1 live).
            stats = big.tile([128, 2 * (NT + 1)], f32, tag="stats")
            nc.vector.memset(stats[:], 1.0)   # ln(1)=0 on unused lanes
            gb = big.tile([128, NT + 1], bf16, tag="gb")
            nc.vector.memset(gb[:], 0.0)

            # ---- gather flat indices: ids + host-staged row-base iota ----
            flat_all = sm.tile([128, NT + 1], i32, tag="flat")
            nc.vector.tensor_tensor(
                out=flat_all[:], in0=ids_all[:],
                in1=blob_sb[:, FLATB0:FLATB0 + NT + 1].bitcast(i32),
                op=Alu.add)
            for t in range(NT + 1):
                # HW DGE honors one index per partition per transfer
                nc.gpsimd.indirect_dma_start(
                    out=gb[:, t:t + 1], out_offset=None, in_=lg_flat,
                    in_offset=bass.IndirectOffsetOnAxis(
                        ap=flat_all[:, t:t + 1], axis=0))

            # ---- conv: 80 matmuls accumulate into one PSUM bank ----
            psum_y = ps_y.tile([OC, Y_LOC], f32, tag="y")
            first = True
            for cc in range(16):
                for k in range(K):
                    nc.tensor.matmul(
                        out=psum_y[:],
                        lhsT=wtile[:, cc * 640 + k * 128: cc * 640 + (k + 1) * 128],
                        rhs=xt[:, cc * EXT + k: cc * EXT + k + Y_LOC],
                        start=first,
                        stop=False,
                    )
                    first = False

            # ---- main pass: per row tile, 8 vocab chunks split scalar/DVE --
            for t in range(NT):
                r0 = 128 * t
                sums = sm.tile([128, NCH], f32, tag=f"sums{t}")
                for ci in range(NCH):
                    x_sb = lp.tile([128, CF], bf16, tag="x")
                    nc.sync.dma_start(
                        out=x_sb[:],
                        in_=lg[r0:r0 + 128, ci * CF:(ci + 1) * CF],
                    )
                    if ci in SCAL_CH:
                        e_sb = eo.tile([128, CF], bf16, tag="e")
                        nc.scalar.activation(
                            out=e_sb[:], in_=x_sb[:], func=Act.Exp,
                            accum_out=sums[:, ci:ci + 1],
                        )
                    else:
                        q_sb = si.tile([128, CF], i16, tag="q")
                        nc.vector.tensor_scalar(
                            out=q_sb[:], in0=x_sb[:], scalar1=A16,
                            scalar2=B16C, op0=Alu.mult, op1=Alu.add)
                        nc.vector.tensor_reduce(
                            out=sums[:, ci:ci + 1], in_=q_sb[:].bitcast(bf16),
                            axis=mybir.AxisListType.X, op=Alu.add)
                nc.vector.tensor_reduce(
                    out=stats[:, 2 * t:2 * t + 1], in_=sums[:],
                    axis=mybir.AxisListType.X, op=Alu.add)

            # ---- halo rows (2): vocab packed across partitions ----
            hx = lp.tile([128, HF], bf16, tag="x")
            halo_src = bass.AP(lg.tensor, 512 * V,
                               [[V, NHALO], [HF, HQ], [1, HF]])
            nc.sync.dma_start(out=hx[:], in_=halo_src)
            he = eo.tile([128, HF], bf16, tag="e")
            hsums = sm.tile([128, 1], f32, tag="hsums")
            nc.scalar.activation(out=he[:], in_=hx[:], func=Act.Exp,
                                 accum_out=hsums[:])
            hsel = big.tile([128, NHALO], f32, tag="hsel")
            nc.vector.memset(hsel[:], 0.0)
            for a in range(NHALO):
                nc.vector.memset(hsel[a * HQ:(a + 1) * HQ, a:a + 1], 1.0)
            psum_h = ps_o.tile([NHALO, 1], f32, tag="ph")
            nc.tensor.matmul(out=psum_h[:], lhsT=hsel[:], rhs=hsums[:],
                             start=True, stop=True)
            nc.vector.tensor_copy(out=stats[:NHALO, 2 * NT:2 * NT + 1],
                                  in_=psum_h[:])

            # ---- gathered logits -> stats odd cols (f32) ----
            nc.vector.tensor_copy(out=stats[:, 1:2 * (NT + 1):2], in_=gb[:])

            # ---- batched surp: (ln(se) - g) * mask  [128, 5] ----
            lse_all = sm.tile([128, NT + 1], f32, tag="lse")
            nc.scalar.activation(out=lse_all[:], in_=stats[:, 0:2 * (NT + 1):2],
                                 func=Act.Ln)
            surp_all = sm.tile([128, NT + 1], f32, tag="surp")
            nc.vector.tensor_tensor(out=surp_all[:], in0=lse_all[:],
                                    in1=stats[:, 1:2 * (NT + 1):2],
                                    op=Alu.subtract)
            nc.vector.tensor_tensor(out=surp_all[:], in0=surp_all[:],
                                    in1=blob_sb[:, MASKC0:MASKC0 + NT + 1],
                                    op=Alu.mult)

            # ---- transpose surp to one row [1, EXT] via TensorE (bf16) ----
            surp_b = sm.tile([128, NT + 1], bf16, tag="surpb")
            nc.vector.tensor_copy(out=surp_b[:], in_=surp_all[:])
            srow = big.tile([1, EXT], bf16, tag="srow")
            for t in range(NT):
                tp = ps_t.tile([128, 128], bf16, tag="tp")
                nc.tensor.transpose(out=tp[:1, :128], in_=surp_b[:, t:t + 1],
                                    identity=identb[:, :])
                nc.vector.tensor_copy(out=srow[0:1, 128 * t:128 * (t + 1)],
                                      in_=tp[0:1, :128])
            tp = ps_t.tile([128, 128], bf16, tag="tp")
            nc.tensor.transpose(out=tp[:1, :NHALO],
                                in_=surp_b[:NHALO, NT:NT + 1],
                                identity=identb[:NHALO, :NHALO])
            nc.vector.tensor_copy(out=srow[0:1, 512:EXT], in_=tp[0:1, :NHALO])

            # ---- surp channel: 5 rank-1 matmuls close the accumulation ----
            WS0 = 16 * K * OC
            for k in range(K):
                nc.tensor.matmul(
                    out=psum_y[:],
                    lhsT=wtile[0:1, WS0 + k * OC: WS0 + (k + 1) * OC],
                    rhs=srow[0:1, k:k + Y_LOC],
                    start=False,
                    stop=(k == K - 1),
                )

            # ---- maxpool(5) + bias + relu ----
            pooled = big.tile([OC, PO_LOC], f32, tag="pooled")
            stop_off = K * (PO_LOC - 1) + 1
            nc.vector.tensor_copy(out=pooled[:], in_=psum_y[:, 0:stop_off:K])
            for j in range(1, K):
                nc.vector.tensor_tensor(out=pooled[:], in0=pooled[:],
                                        in1=psum_y[:, j:j + stop_off:K],
                                        op=Alu.max)
            nc.vector.tensor_scalar(out=pooled[:], in0=pooled[:],
                                    scalar1=blob_sb[:, CONVB0:CONVB0 + 1],
                                    scalar2=0.0, op0=Alu.add, op1=Alu.max)

            # ---- FC partial: red[oc, l] = sum_j pooled*fcw ----
            red = big.tile([OC, 3], f32, tag="red")
            fc_scr = big.tile([OC, PO_LOC], f32, tag="fcscr")
            for l in range(3):
                nc.vector.tensor_tensor(
                    out=fc_scr[:],
                    in0=pooled[:],
                    in1=blob_sb[:, FCW0 + l * PO_LOC: FCW0 + (l + 1) * PO_LOC],
                    op=Alu.mult,
                )
                nc.vector.tensor_reduce(
                    out=red[:, l:l + 1], in_=fc_scr[:],
                    axis=mybir.AxisListType.X, op=Alu.add,
                )
            # sentiment branch (zeroed on h==1 cores)
            rs = sm.tile([128, 1], f32, tag="rs")
            nc.vector.tensor_scalar(out=rs[:], in0=blob_sb[:, SENTV0:SENTV0 + 1],
                                    scalar1=0.0, scalar2=None, op0=Alu.max)
            tmp3 = sm.tile([128, 3], f32, tag="tmp3")
            nc.vector.tensor_scalar(out=tmp3[:],
                                    in0=blob_sb[:, SENTW0:SENTW0 + 3],
                                    scalar1=rs[:, 0:1], scalar2=None,
                                    op0=Alu.mult)
            nc.vector.tensor_tensor(out=red[:], in0=red[:], in1=tmp3[:],
                                    op=Alu.add)

            psum_out = ps_o.tile([3, 1], f32, tag="po")
            nc.tensor.matmul(out=psum_out[:], lhsT=red[:], rhs=ones_sb[:],
                             start=True, stop=True)
            out_sb = sm.tile([3, 1], f32, tag="outsb")
            nc.vector.tensor_tensor(out=out_sb[:], in0=psum_out[:],
                                    in1=blob_sb[0:3, FCB0:FCB0 + 1],
                                    op=Alu.add)
            nc.sync.dma_start(out=out, in_=out_sb[:])

    nc.compile()
    return nc


# blob column layout (f32 [128, BS])
MASKC0 = 0                      # [128, NT+1] attention mask, col-major tiles
FLATB0 = MASKC0 + NT + 1        # [128, NT+1] gather row-base (i32 bits)
CONVB0 = FLATB0 + NT + 1        # [OC, 1] conv bias
FCW0 = CONVB0 + 1               # [OC, 3*PO_LOC] fc weights for this shard
SENTV0 = FCW0 + 3 * PO_LOC      # [3, 1] sentiment values (h==0 only)
SENTW0 = SENTV0 + 1             # [3, 3] fc weights for sentiment cols
FCB0 = SENTW0 + 3               # [3, 1] fc bias (h==0 only)
BS = FCB0 + 1


def _prep_core_inputs(core, input_ids, attention_mask, sentiment, logits,
                      hidden, conv_w, conv_b, fc_w, fc_b, bf16):
    b, h = core // 2, core % 2
    g0 = Y_LOC * h
    ext0 = g0 - 2
    lo = max(0, -ext0)            # local index where valid rows start
    s0, s1 = ext0 + lo, ext0 + EXT

    lg = np.zeros((EXT, V), bf16)
    lg[lo:] = logits[b, s0:s1].astype(bf16)
    idl = np.zeros((128 * (NT + 1), 1), np.int32)
    idl[lo:EXT, 0] = input_ids[b, s0:s1].astype(np.int32)

    mask_ext = np.zeros((EXT,), np.float32)
    mask_ext[lo:] = attention_mask[b, s0:s1]

    # hidden, transposed to [128, 16*EXT]: col cc*EXT+j = hidden[s, cc*128+p]
    hd = np.zeros((EXT, H), np.float32)
    hd[lo:] = hidden[b, s0:s1]
    hidt = np.ascontiguousarray(
        hd.T.astype(bf16).reshape(16, 128, EXT).transpose(1, 0, 2)
        .reshape(128, 16 * EXT))

    # conv weights [128, 17*640]: wt16[p, cc*640 + k*128 + oc] = w[oc, cc*128+p, k]
    # last 640 cols: partition 0 = surp-channel weights * LOG2E
    wt = conv_w[:, :H, :].transpose(1, 2, 0).reshape(16, 128, K * OC)
    wt16 = np.zeros((128, 17 * K * OC), bf16)
    wt16[:, :16 * K * OC] = wt.transpose(1, 0, 2).reshape(128, 16 * K * OC)
    wt16[0, 16 * K * OC:] = (conv_w[:, H, :].T * LOG2E).reshape(-1)

    blob = np.zeros((128, BS), np.float32)
    mcol = np.zeros((128, NT + 1), np.float32)
    mcol[:, :NT] = mask_ext[:512].reshape(NT, 128).T
    mcol[:NHALO, NT] = mask_ext[512:]
    blob[:, MASKC0:MASKC0 + NT + 1] = mcol
    fb = np.zeros((128, NT + 1), np.int32)
    p = np.arange(128)
    for t in range(NT):
        fb[:, t] = (128 * t + p) * V
    fb[:NHALO, NT] = (512 + np.arange(NHALO)) * V
    blob[:, FLATB0:FLATB0 + NT + 1] = fb.view(np.float32)
    blob[:, CONVB0] = conv_b
    w3 = fc_w[:, :OC * 204].reshape(3, OC, 204)
    blob[:, FCW0:FCW0 + 3 * PO_LOC] = np.ascontiguousarray(
        w3[:, :, h * PO_LOC:(h + 1) * PO_LOC].transpose(1, 0, 2)
        .reshape(OC, 3 * PO_LOC))
    if h == 0:
        blob[:3, SENTV0] = sentiment[b]
        blob[:3, SENTW0:SENTW0 + 3] = fc_w[:, OC * 204:].T
        blob[:3, FCB0] = fc_b

    return {"lg16": lg, "hidt": hidt, "wt16": wt16, "ids": idl, "blob": blob}


def _install_ntff_hook():
    import sys
    import types
    try:
        import antenv
        from trn_agent_boot.trn_boot import _ntff_profile_via_ctypes
    except ImportError:
        return
    if "antenv.axon_hooks" in sys.modules:
        return
    mod = types.ModuleType("antenv.axon_hooks")
    _h = [None]
    mod.set_axon_ntff_profile_hook = lambda hk: _h.__setitem__(0, hk)
    mod.get_axon_ntff_profile_hook = lambda: _h[0]
    sys.modules["antenv.axon_hooks"] = mod
    antenv.axon_hooks = mod
    try:
        mod.set_axon_ntff_profile_hook(
            _ntff_profile_via_ctypes('/opt/axon/libaxon_pjrt.so'))
    except Exception:
        pass


def kernel(input_ids, attention_mask, sentiment, logits, hidden,
           conv_w, conv_b, fc_w, fc_b, _trace=False):
    import ml_dtypes
    from concourse.bass_utils import run_bass_kernel_spmd

    bf16 = ml_dtypes.bfloat16
    input_ids = np.asarray(input_ids)
    attention_mask = np.asarray(attention_mask, np.float32)
    sentiment = np.asarray(sentiment, np.float32)
    logits = np.asarray(logits, np.float32)
    hidden = np.asarray(hidden, np.float32)
    conv_w = np.asarray(conv_w, np.float32)
    conv_b = np.asarray(conv_b, np.float32)
    fc_w = np.asarray(fc_w, np.float32)
    fc_b = np.asarray(fc_b, np.float32)

    if "nc" not in _CACHE:
        _CACHE["nc"] = _build_program()
    nc = _CACHE["nc"]

    in_maps = [
        _prep_core_inputs(c, input_ids, attention_mask, sentiment, logits,
                          hidden, conv_w, conv_b, fc_w, fc_b, bf16)
        for c in range(N_CORES)
    ]
    if _trace:
        _install_ntff_hook()
    res = run_bass_kernel_spmd(nc, in_maps, list(range(N_CORES)), trace=_trace)
    _CACHE["last_result"] = res

    out = np.zeros((B, 3), np.float32)
    for b in range(B):
        out[b] = (res.results[2 * b]["out_loc"][:, 0]
                  + res.results[2 * b + 1]["out_loc"][:, 0])
    return out


# revision 9
# speedup vs baseline: 1.8754x; 1.0015x over previous
"""Trainium2 Bass kernel for nn_CNN_80221399155117.

Pipeline: full-vocab softmax -> token-prob gather -> -log2 surprisal ->
concat(hidden, surp) -> Conv1d(k=5, pad=2) -> MaxPool1d(5) -> ReLU -> FC.

Sharding: 8 cores = (batch b, seq-half h). Each core owns the pool-aligned
conv-output range [510h, 510h+510) of its batch, needing feats rows
[510h-2, 510h+512) (EXT=514, zero-padded outside [0,1024)). The softmax
normalizer is computed locally per row (positions sharded, vocab local).

Perf structure:
- logits staged bf16 (halves HBM traffic); hidden host-transposed bf16.
- vocab chunks split between ScalarE (exact EXP + accum) and DVE
  (Schraudolph fast-exp: i16 = round(A*x + B) -> bitcast bf16 -> reduce;
  both ops run in the DVE 4x perf mode). The ~1.8% per-element error is
  bias-corrected in B and averages out in the 32000-term softmax sum.
- token-logit gather via indirect DMA (exact bf16 logit).
- conv as 80 accumulated matmuls vs resident transposed activations; the
  surprisal channel closes the accumulation with 5 rank-1 matmuls against
  the assembled surp row (no DMA round-trips in the epilogue).
"""

import numpy as np

B, S, V, H = 4, 1024, 32000, 2048
OC, K = 128, 5
N_CORES = 8
Y_LOC = 510            # conv output positions per core (102 pool windows)
PO_LOC = 102           # pooled cols per core
EXT = 514              # feats rows incl conv halo (510 + 2 + 2)
CF = 4000              # vocab chunk (free-dim) size
NCH = V // CF          # 8 chunks
NT = 4                 # main row tiles of 128
NHALO = EXT - 512      # 2 halo rows, packed [128, HF]
HQ = 128 // NHALO      # partitions per halo row
HF = V // HQ           # free elems per partition
LOG2E = 1.4426950408889634

A16 = 184.6650030622249        # 2^7 / ln 2
B16C = 16248.638470970125      # 127*2^7 + 0.5ulp-tuned bias correction
SCAL_CH = (3, 4, 5, 6, 7)      # chunks on ScalarE (exact exp)

_CACHE = {}


def _build_program():
    import concourse.tile as tile
    from concourse import bacc, bass, mybir
    from concourse.masks import make_identity

    f32 = mybir.dt.float32
    bf16 = mybir.dt.bfloat16
    i16 = mybir.dt.int16
    i32 = mybir.dt.int32
    Alu = mybir.AluOpType
    Act = mybir.ActivationFunctionType

    nc = bacc.Bacc("TRN2", target_bir_lowering=False, debug=False,
                   num_devices=N_CORES)

    lg = nc.dram_tensor("lg16", [EXT, V], bf16, kind="ExternalInput").ap()
    hidt = nc.dram_tensor("hidt", [128, 16 * EXT], bf16, kind="ExternalInput").ap()
    wt16 = nc.dram_tensor("wt16", [128, 17 * K * OC], bf16,
                          kind="ExternalInput").ap()
    ids = nc.dram_tensor("ids", [128 * (NT + 1), 1], i32,
                         kind="ExternalInput").ap()
    blob = nc.dram_tensor("blob", [128, BS], f32, kind="ExternalInput").ap()
    out = nc.dram_tensor("out_loc", [3, 1], f32, kind="ExternalOutput").ap()

    lg_flat = bass.AP(lg.tensor, 0, [[1, EXT * V], [1, 1]])

    with tile.TileContext(nc) as tc:
        with (
            tc.tile_pool(name="lp", bufs=8) as lp,          # logits chunks
            tc.tile_pool(name="eo", bufs=2) as eo,          # scalar exp out
            tc.tile_pool(name="si", bufs=3) as si,          # dve i16 bits
            tc.tile_pool(name="big", bufs=1) as big,        # resident
            tc.tile_pool(name="sm", bufs=10) as sm,         # small tiles
            tc.tile_pool(name="ps_t", bufs=2, space="PSUM") as ps_t,
            tc.tile_pool(name="ps_y", bufs=1, space="PSUM") as ps_y,
            tc.tile_pool(name="ps_o", bufs=2, space="PSUM") as ps_o,
        ):
            # ---- small residents first on the sync ring (feed the gathers
            # during the DMA ramp); bulky xt/wtile deferred to the ACT ring --
            blob_sb = big.tile([128, BS], f32, tag="blob")
            nc.sync.dma_start(out=blob_sb[:], in_=blob)
            ids_all = sm.tile([128, NT + 1], i32, tag="ids")
            nc.sync.dma_start(
                out=ids_all[:],
                in_=bass.AP(ids.tensor, 0, [[1, 128], [128, NT + 1]]))
            xt = big.tile([128, 16 * EXT], bf16, tag="xt")
            wtile = big.tile([128, 17 * K * OC], bf16, tag="wtile")

            identb = big.tile([128, 128], bf16, tag="identb")
            make_identity(nc, identb[:])
            ones_sb = big.tile([128, 1], f32, tag="ones")
            nc.vector.memset(ones_sb[:], 1.0)

            # stats [128, 10]: col 2t = sum(exp) tile t, col 2t+1 = gathered
            # logit; t=4 is the halo slab (partitions 0-1 live).
            stats = big.tile([128, 2 * (NT + 1)], f32, tag="stats")
            nc.vector.memset(stats[:], 1.0)   # ln(1)=0 on unused lanes
            gb = big.tile([128, NT + 1], bf16, tag="gb")
            nc.vector.memset(gb[:], 0.0)

            # ---- gather flat indices: ids + host-staged row-base iota ----
            flat_all = sm.tile([128, NT + 1], i32, tag="flat")
            nc.vector.tensor_tensor(
                out=flat_all[:], in0=ids_all[:],
                in1=blob_sb[:, FLATB0:FLATB0 + NT + 1].bitcast(i32),
                op=Alu.add)
            for t in range(NT + 1):
                # HW DGE honors one index per partition per transfer
                nc.gpsimd.indirect_dma_start(
                    out=gb[:, t:t + 1], out_offset=None, in_=lg_flat,
                    in_offset=bass.IndirectOffsetOnAxis(
                        ap=flat_all[:, t:t + 1], axis=0))

            # ---- main pass: per row tile, 8 vocab chunks split scalar/DVE --
            for t in range(NT):
                if t == 1:
                    # bulk residents ride the idle ACT ring once chunk DMAs
                    # own the sync ring
                    nc.scalar.dma_start(out=xt[:], in_=hidt)
                    nc.scalar.dma_start(out=wtile[:], in_=wt16)
                r0 = 128 * t
                sums = sm.tile([128, NCH], f32, tag=f"sums{t}")
                for ci in range(NCH):
                    x_sb = lp.tile([128, CF], bf16, tag="x")
                    nc.sync.dma_start(
                        out=x_sb[:],
                        in_=lg[r0:r0 + 128, ci * CF:(ci + 1) * CF],
                    )
                    if ci in SCAL_CH:
                        e_sb = eo.tile([128, CF], bf16, tag="e")
                        nc.scalar.activation(
                            out=e_sb[:], in_=x_sb[:], func=Act.Exp,
                            accum_out=sums[:, ci:ci + 1],
                        )
                    else:
                        q_sb = si.tile([128, CF], i16, tag="q")
                        nc.vector.tensor_scalar(
                            out=q_sb[:], in0=x_sb[:], scalar1=A16,
                            scalar2=B16C, op0=Alu.mult, op1=Alu.add)
                        nc.vector.tensor_reduce(
                            out=sums[:, ci:ci + 1], in_=q_sb[:].bitcast(bf16),
                            axis=mybir.AxisListType.X, op=Alu.add)
                nc.vector.tensor_reduce(
                    out=stats[:, 2 * t:2 * t + 1], in_=sums[:],
                    axis=mybir.AxisListType.X, op=Alu.add)

            # ---- conv: 80 matmuls accumulate into one PSUM bank ----
            psum_y = ps_y.tile([OC, Y_LOC], f32, tag="y")
            first = True
            for cc in range(16):
                for k in range(K):
                    nc.tensor.matmul(
                        out=psum_y[:],
                        lhsT=wtile[:, cc * 640 + k * 128: cc * 640 + (k + 1) * 128],
                        rhs=xt[:, cc * EXT + k: cc * EXT + k + Y_LOC],
                        start=first,
                        stop=False,
                    )
                    first = False

            # ---- halo rows (2): vocab packed across partitions ----
            hx = lp.tile([128, HF], bf16, tag="x")
            halo_src = bass.AP(lg.tensor, 512 * V,
                               [[V, NHALO], [HF, HQ], [1, HF]])
            nc.sync.dma_start(out=hx[:], in_=halo_src)
            he = eo.tile([128, HF], bf16, tag="e")
            hsums = sm.tile([128, 1], f32, tag="hsums")
            nc.scalar.activation(out=he[:], in_=hx[:], func=Act.Exp,
                                 accum_out=hsums[:])
            hsel = big.tile([128, NHALO], f32, tag="hsel")
            nc.vector.memset(hsel[:], 0.0)
            for a in range(NHALO):
                nc.vector.memset(hsel[a * HQ:(a + 1) * HQ, a:a + 1], 1.0)
            psum_h = ps_o.tile([NHALO, 1], f32, tag="ph")
            nc.tensor.matmul(out=psum_h[:], lhsT=hsel[:], rhs=hsums[:],
                             start=True, stop=True)
            nc.vector.tensor_copy(out=stats[:NHALO, 2 * NT:2 * NT + 1],
                                  in_=psum_h[:])

            # ---- gathered logits -> stats odd cols (f32) ----
            nc.vector.tensor_copy(out=stats[:, 1:2 * (NT + 1):2], in_=gb[:])

            # ---- batched surp: (ln(se) - g) * mask  [128, 5] ----
            lse_all = sm.tile([128, NT + 1], f32, tag="lse")
            nc.scalar.activation(out=lse_all[:], in_=stats[:, 0:2 * (NT + 1):2],
                                 func=Act.Ln)
            surp_all = sm.tile([128, NT + 1], f32, tag="surp")
            nc.vector.tensor_tensor(out=surp_all[:], in0=lse_all[:],
                                    in1=stats[:, 1:2 * (NT + 1):2],
                                    op=Alu.subtract)
            nc.vector.tensor_tensor(out=surp_all[:], in0=surp_all[:],
                                    in1=blob_sb[:, MASKC0:MASKC0 + NT + 1],
                                    op=Alu.mult)

            # ---- transpose surp to one row [1, EXT] via TensorE (bf16) ----
            surp_b = sm.tile([128, NT + 1], bf16, tag="surpb")
            nc.vector.tensor_copy(out=surp_b[:], in_=surp_all[:])
            srow = big.tile([1, EXT], bf16, tag="srow")
            for t in range(NT):
                tp = ps_t.tile([128, 128], bf16, tag="tp")
                nc.tensor.transpose(out=tp[:1, :128], in_=surp_b[:, t:t + 1],
                                    identity=identb[:, :])
                nc.vector.tensor_copy(out=srow[0:1, 128 * t:128 * (t + 1)],
                                      in_=tp[0:1, :128])
            tp = ps_t.tile([128, 128], bf16, tag="tp")
            nc.tensor.transpose(out=tp[:1, :NHALO],
                                in_=surp_b[:NHALO, NT:NT + 1],
                                identity=identb[:NHALO, :NHALO])
            nc.vector.tensor_copy(out=srow[0:1, 512:EXT], in_=tp[0:1, :NHALO])

            # ---- surp channel: 5 rank-1 matmuls close the accumulation ----
            WS0 = 16 * K * OC
            for k in range(K):
                nc.tensor.matmul(
                    out=psum_y[:],
                    lhsT=wtile[0:1, WS0 + k * OC: WS0 + (k + 1) * OC],
                    rhs=srow[0:1, k:k + Y_LOC],
                    start=False,
                    stop=(k == K - 1),
                )

            # ---- maxpool(5) + bias + relu ----
            pooled = big.tile([OC, PO_LOC], f32, tag="pooled")
            stop_off = K * (PO_LOC - 1) + 1
            nc.vector.tensor_copy(out=pooled[:], in_=psum_y[:, 0:stop_off:K])
            for j in range(1, K):
                nc.vector.tensor_tensor(out=pooled[:], in0=pooled[:],
                                        in1=psum_y[:, j:j + stop_off:K],
                                        op=Alu.max)
            nc.vector.tensor_scalar(out=pooled[:], in0=pooled[:],
                                    scalar1=blob_sb[:, CONVB0:CONVB0 + 1],
                                    scalar2=0.0, op0=Alu.add, op1=Alu.max)

            # ---- FC partial: red[oc, l] = sum_j pooled*fcw ----
            red = big.tile([OC, 3], f32, tag="red")
            fc_scr = big.tile([OC, PO_LOC], f32, tag="fcscr")
            for l in range(3):
                nc.vector.tensor_tensor(
                    out=fc_scr[:],
                    in0=pooled[:],
                    in1=blob_sb[:, FCW0 + l * PO_LOC: FCW0 + (l + 1) * PO_LOC],
                    op=Alu.mult,
                )
                nc.vector.tensor_reduce(
                    out=red[:, l:l + 1], in_=fc_scr[:],
                    axis=mybir.AxisListType.X, op=Alu.add,
                )
            # sentiment branch (zeroed on h==1 cores)
            rs = sm.tile([128, 1], f32, tag="rs")
            nc.vector.tensor_scalar(out=rs[:], in0=blob_sb[:, SENTV0:SENTV0 + 1],
                                    scalar1=0.0, scalar2=None, op0=Alu.max)
            tmp3 = sm.tile([128, 3], f32, tag="tmp3")
            nc.vector.tensor_scalar(out=tmp3[:],
                                    in0=blob_sb[:, SENTW0:SENTW0 + 3],
                                    scalar1=rs[:, 0:1], scalar2=None,
                                    op0=Alu.mult)
            nc.vector.tensor_tensor(out=red[:], in0=red[:], in1=tmp3[:],
                                    op=Alu.add)

            psum_out = ps_o.tile([3, 1], f32, tag="po")
            nc.tensor.matmul(out=psum_out[:], lhsT=red[:], rhs=ones_sb[:],
                             start=True, stop=True)
            out_sb = sm.tile([3, 1], f32, tag="outsb")
            nc.vector.tensor_tensor(out=out_sb[:], in0=psum_out[:],
                                    in1=blob_sb[0:3, FCB0:FCB0 + 1],
                                    op=Alu.add)
            nc.sync.dma_start(out=out, in_=out_sb[:])

    nc.compile()
    return nc


# blob column layout (f32 [128, BS])
MASKC0 = 0                      # [128, NT+1] attention mask, col-major tiles
FLATB0 = MASKC0 + NT + 1        # [128, NT+1] gather row-base (i32 bits)
CONVB0 = FLATB0 + NT + 1        # [OC, 1] conv bias
FCW0 = CONVB0 + 1               # [OC, 3*PO_LOC] fc weights for this shard
SENTV0 = FCW0 + 3 * PO_LOC      # [3, 1] sentiment values (h==0 only)
SENTW0 = SENTV0 + 1             # [3, 3] fc weights for sentiment cols
FCB0 = SENTW0 + 3               # [3, 1] fc bias (h==0 only)
BS = FCB0 + 1


def _prep_core_inputs(core, input_ids, attention_mask, sentiment, logits,
                      hidden, conv_w, conv_b, fc_w, fc_b, bf16):
    b, h = core // 2, core % 2
    g0 = Y_LOC * h
    ext0 = g0 - 2
    lo = max(0, -ext0)            # local index where valid rows start
    s0, s1 = ext0 + lo, ext0 + EXT

    lg = np.zeros((EXT, V), bf16)
    lg[lo:] = logits[b, s0:s1].astype(bf16)
    idl = np.zeros((128 * (NT + 1), 1), np.int32)
    idl[lo:EXT, 0] = input_ids[b, s0:s1].astype(np.int32)

    mask_ext = np.zeros((EXT,), np.float32)
    mask_ext[lo:] = attention_mask[b, s0:s1]

    # hidden, transposed to [128, 16*EXT]: col cc*EXT+j = hidden[s, cc*128+p]
    hd = np.zeros((EXT, H), np.float32)
    hd[lo:] = hidden[b, s0:s1]
    hidt = np.ascontiguousarray(
        hd.T.astype(bf16).reshape(16, 128, EXT).transpose(1, 0, 2)
        .reshape(128, 16 * EXT))

    # conv weights [128, 17*640]: wt16[p, cc*640 + k*128 + oc] = w[oc, cc*128+p, k]
    # last 640 cols: partition 0 = surp-channel weights * LOG2E
    wt = conv_w[:, :H, :].transpose(1, 2, 0).reshape(16, 128, K * OC)
    wt16 = np.zeros((128, 17 * K * OC), bf16)
    wt16[:, :16 * K * OC] = wt.transpose(1, 0, 2).reshape(128, 16 * K * OC)
    wt16[0, 16 * K * OC:] = (conv_w[:, H, :].T * LOG2E).reshape(-1)

    blob = np.zeros((128, BS), np.float32)
    mcol = np.zeros((128, NT + 1), np.float32)
    mcol[:, :NT] = mask_ext[:512].reshape(NT, 128).T
    mcol[:NHALO, NT] = mask_ext[512:]
    blob[:, MASKC0:MASKC0 + NT + 1] = mcol
    fb = np.zeros((128, NT + 1), np.int32)
    p = np.arange(128)
    for t in range(NT):
        fb[:, t] = (128 * t + p) * V
    fb[:NHALO, NT] = (512 + np.arange(NHALO)) * V
    blob[:, FLATB0:FLATB0 + NT + 1] = fb.view(np.float32)
    blob[:, CONVB0] = conv_b
    w3 = fc_w[:, :OC * 204].reshape(3, OC, 204)
    blob[:, FCW0:FCW0 + 3 * PO_LOC] = np.ascontiguousarray(
        w3[:, :, h * PO_LOC:(h + 1) * PO_LOC].transpose(1, 0, 2)
        .reshape(OC, 3 * PO_LOC))
    if h == 0:
        blob[:3, SENTV0] = sentiment[b]
        blob[:3, SENTW0:SENTW0 + 3] = fc_w[:, OC * 204:].T
        blob[:3, FCB0] = fc_b

    return {"lg16": lg, "hidt": hidt, "wt16": wt16, "ids": idl, "blob": blob}


def _install_ntff_hook():
    import sys
    import types
    try:
        import antenv
        from trn_agent_boot.trn_boot import _ntff_profile_via_ctypes
    except ImportError:
        return
    if "antenv.axon_hooks" in sys.modules:
        return
    mod = types.ModuleType("antenv.axon_hooks")
    _h = [None]
    mod.set_axon_ntff_profile_hook = lambda hk: _h.__setitem__(0, hk)
    mod.get_axon_ntff_profile_hook = lambda: _h[0]
    sys.modules["antenv.axon_hooks"] = mod
    antenv.axon_hooks = mod
    try:
        mod.set_axon_ntff_profile_hook(
            _ntff_profile_via_ctypes('/opt/axon/libaxon_pjrt.so'))
    except Exception:
        pass


def kernel(input_ids, attention_mask, sentiment, logits, hidden,
           conv_w, conv_b, fc_w, fc_b, _trace=False):
    import ml_dtypes
    from concourse.bass_utils import run_bass_kernel_spmd

    bf16 = ml_dtypes.bfloat16
    input_ids = np.asarray(input_ids)
    attention_mask = np.asarray(attention_mask, np.float32)
    sentiment = np.asarray(sentiment, np.float32)
    logits = np.asarray(logits, np.float32)
    hidden = np.asarray(hidden, np.float32)
    conv_w = np.asarray(conv_w, np.float32)
    conv_b = np.asarray(conv_b, np.float32)
    fc_w = np.asarray(fc_w, np.float32)
    fc_b = np.asarray(fc_b, np.float32)

    if "nc" not in _CACHE:
        _CACHE["nc"] = _build_program()
    nc = _CACHE["nc"]

    in_maps = [
        _prep_core_inputs(c, input_ids, attention_mask, sentiment, logits,
                          hidden, conv_w, conv_b, fc_w, fc_b, bf16)
        for c in range(N_CORES)
    ]
    if _trace:
        _install_ntff_hook()
    res = run_bass_kernel_spmd(nc, in_maps, list(range(N_CORES)), trace=_trace)
    _CACHE["last_result"] = res

    out = np.zeros((B, 3), np.float32)
    for b in range(B):
        out[b] = (res.results[2 * b]["out_loc"][:, 0]
                  + res.results[2 * b + 1]["out_loc"][:, 0])
    return out
